# revision 17
# baseline (speedup 1.0000x reference)
"""Trainium2 Bass kernel for LocalGlobalEnvEncoder (GCN + MHA fusion).

Sharding: nodes are split across the 8 cores (1024 dest nodes / queries each).
 - GCN: edges bucketed by destination node-tile on host (layout only); degrees
   computed on-device via one-hot matmuls, exchanged with an AllGather; messages
   gathered from a device-materialized y = x * rsqrt(d) table via batched
   indirect DMA and scatter-added with one-hot matmuls on the PE.
 - MHA: query-sharded attention, K/V computed redundantly per core, scores kept
   transposed ([key, query]); attn@V accumulates O^T per head with an appended
   ones-column in V providing softmax denominators.
Region A: degrees + QKV projections (PE busy while indices load).
Region B: AllGather + y table + attention + interleaved GCN scatter.
Region C: out_proj + LN + combine + fc.
All heavy matmuls run in bf16 (fp32 PSUM accumulation); normalization fp32.
"""
import sys
sys.path.insert(0, '/opt/trn_rl_repo')
import numpy as np
import ml_dtypes
import concourse.bass as bass
import concourse.tile as tile
from concourse import bacc, mybir, library_config
from concourse.bass_utils import run_bass_kernel_spmd

F32 = mybir.dt.float32
BF16 = mybir.dt.bfloat16
I32 = mybir.dt.int32
AF = mybir.ActivationFunctionType
OP = mybir.AluOpType
AX = mybir.AxisListType

N, E, C, OUTC, H, DH = 8192, 262144, 256, 256, 4, 64
NCORES = 8
NPC = N // NCORES          # nodes per core = 1024
P = 128
NT_LOC = NPC // P          # node tiles per core = 8
NT_GLOB = N // P           # global node tiles = 64
EXP_BIAS = -12.0           # uniform shift inside softmax exp; cancels in the ratio
GATHER_GRP = 8             # edge tiles per indirect-DMA gather (1024 descriptors)

LAST_RESULTS = None        # stashed BassKernelResults for test harness introspection


def _build(TPT):
    """Build the single SPMD Bass program. TPT = edge tiles per node-tile segment."""
    nc = bacc.Bacc('TRN2', target_bir_lowering=False, debug=False, num_devices=NCORES)
    TE = NT_LOC * TPT  # total edge tiles per core

    # ---- I/O ----
    xT = nc.dram_tensor("xT", [C, N], BF16, kind="ExternalInput")
    xT_own = nc.dram_tensor("xT_own", [C, NPC], BF16, kind="ExternalInput")
    x_full = nc.dram_tensor("x_full", [N, C], BF16, kind="ExternalInput")
    x_own = nc.dram_tensor("x_own", [NPC, C], F32, kind="ExternalInput")
    WqT = nc.dram_tensor("WqT", [C, C], BF16, kind="ExternalInput")
    WkT = nc.dram_tensor("WkT", [C, C], BF16, kind="ExternalInput")
    WvT = nc.dram_tensor("WvT", [C, C], BF16, kind="ExternalInput")
    Wop4 = nc.dram_tensor("Wop4", [DH, H * C], BF16, kind="ExternalInput")
    Wl = nc.dram_tensor("Wl", [C, C], BF16, kind="ExternalInput")
    fcT = nc.dram_tensor("fcT", [C, OUTC], BF16, kind="ExternalInput")
    bq_pack = nc.dram_tensor("bq_pack", [P, 2], F32, kind="ExternalInput")
    bk_pack = nc.dram_tensor("bk_pack", [P, 2], F32, kind="ExternalInput")
    bv_rep = nc.dram_tensor("bv_rep", [P, C], F32, kind="ExternalInput")
    opb_rep = nc.dram_tensor("opb_rep", [P, C], F32, kind="ExternalInput")
    g_rep = nc.dram_tensor("g_rep", [P, C], F32, kind="ExternalInput")
    b_rep = nc.dram_tensor("b_rep", [P, C], F32, kind="ExternalInput")
    fcb_rep = nc.dram_tensor("fcb_rep", [P, OUTC], F32, kind="ExternalInput")
    alpha11 = nc.dram_tensor("alpha11", [1, 1], F32, kind="ExternalInput")
    identb_in = nc.dram_tensor("identb_in", [P, P], BF16, kind="ExternalInput")
    ones_col_in = nc.dram_tensor("ones_col_in", [P, 1], BF16, kind="ExternalInput")
    ones_row_in = nc.dram_tensor("ones_row_in", [1, P], F32, kind="ExternalInput")
    GOH = 4                    # one-hot tiles packed per DMA
    G_CNT = (TE + GOH - 1) // GOH
    oh_in = nc.dram_tensor("oh_in", [G_CNT, P, GOH * P], BF16, kind="ExternalInput")
    row_idx = nc.dram_tensor("row_idx", [P, TE], I32, kind="ExternalInput")

    out = nc.dram_tensor("out", [NPC, OUTC], F32, kind="ExternalOutput")

    with tile.TileContext(nc) as tc:
        with tc.tile_pool(name="const", bufs=1) as const, \
             tc.tile_pool(name="big", bufs=1) as big, \
             tc.tile_pool(name="dram", bufs=1, space="DRAM") as dram:

            # ---- persistent constants ----
            identb_t = const.tile([P, P], BF16)
            nc.sync.dma_start(out=identb_t[:], in_=identb_in[:])
            ones_col_t = const.tile([P, 1], BF16)
            nc.sync.dma_start(out=ones_col_t[:], in_=ones_col_in[:])
            ones_row_t = const.tile([1, P], F32)
            nc.sync.dma_start(out=ones_row_t[:], in_=ones_row_in[:])
            row_t = const.tile([P, TE], I32)
            nc.sync.dma_start(out=row_t[:], in_=row_idx[:])
            expb_col = const.tile([P, 1], F32)
            nc.vector.memset(expb_col[:], EXP_BIAS)
            eps_col = const.tile([P, 1], F32)
            nc.vector.memset(eps_col[:], 1e-5)

            d_loc = const.tile([P, NT_LOC], F32)
            s_own = const.tile([P, NT_LOC], F32)
            s_all = const.tile([P, NT_GLOB], F32)
            w_col = const.tile([P, 1], F32)
            dg_in = dram.tile([NT_LOC, P], F32)
            dg_out = dram.tile([NT_GLOB, P], F32)
            y_d = dram.tile([N, C], BF16)
            den_dr = dram.tile([1, NPC], F32)

            # persistent SBUF state across regions
            KTp = [big.tile([P, N], BF16, name=f"KT{p}") for p in range(2)]
            QTp = [big.tile([P, NPC], BF16, name=f"QT{p}") for p in range(2)]
            Vt = big.tile([P, NT_GLOB * H * (DH + 1)], BF16, name="Vt")
            V4 = Vt[:].rearrange("p (k h d) -> p k h d", h=H, d=DH + 1)
            OTh = [big.tile([DH, NPC], BF16, name=f"OT{h}") for h in range(H)]
            hi_sb = [big.tile([P, C], BF16, name=f"hi{i}") for i in range(NT_LOC)]

            nc.vector.memset(V4[:, :, :, DH:DH + 1], 1.0)  # ones col -> denominators

            # ============ region A: degrees + sigmoid(alpha) + QKV ============
            with tc.tile_pool(name="phA", bufs=1) as phA, \
                 tc.tile_pool(name="psA", bufs=1, space="PSUM") as psA:
                # sigmoid(alpha) replicated to a [128,1] column
                al_t = phA.tile([1, 1], F32)
                nc.sync.dma_start(out=al_t[:], in_=alpha11[:])
                wsig = phA.tile([1, 1], F32)
                nc.scalar.activation(out=wsig[:], in_=al_t[:], func=AF.Sigmoid)
                wrep_ps = psA.tile([P, 1], F32, tag="wrep")
                nc.tensor.matmul(out=wrep_ps[:], lhsT=ones_row_t[:], rhs=wsig[:],
                                 start=True, stop=True)
                nc.vector.tensor_copy(out=w_col[:], in_=wrep_ps[:])

                # degrees of own nodes via one-hot matmuls against a ones column
                ohg_a = {}
                for t in range(NT_LOC):
                    dps = psA.tile([P, 1], F32, tag="dps", bufs=2)
                    for i in range(TPT):
                        j = t * TPT + i
                        if j % GOH == 0:
                            ohg_a[j // GOH] = phA.tile([P, GOH * P], BF16, tag="oh", bufs=6, name=f"ohga{j}")
                            nc.gpsimd.dma_start(out=ohg_a[j // GOH][:], in_=oh_in[j // GOH, :, :])
                        oh = ohg_a[j // GOH][:, (j % GOH) * P:(j % GOH + 1) * P]
                        nc.tensor.matmul(out=dps[:], lhsT=oh, rhs=ones_col_t[:],
                                         start=(i == 0), stop=(i == TPT - 1))
                    nc.vector.tensor_copy(out=d_loc[:, t:t + 1], in_=dps[:])
                    nc.sync.dma_start(out=dg_in[t, :, None], in_=d_loc[:, t:t + 1])

                # launch the degree exchange early; it overlaps the QKV work
                nc.gpsimd.collective_compute(
                    "AllGather", OP.bypass,
                    replica_groups=[list(range(NCORES))],
                    ins=[dg_in[:].opt()], outs=[dg_out[:].opt()])

                # rsqrt for own nodes (s_own); s_all comes after the AllGather
                m_t = phA.tile([P, NT_LOC], F32, tag="mo")
                nc.vector.tensor_scalar(out=m_t[:], in0=d_loc[:], scalar1=1.0,
                                        scalar2=None, op0=OP.min)
                t1 = phA.tile([P, NT_LOC], F32, tag="t1o")
                nc.vector.tensor_scalar(out=t1[:], in0=d_loc[:], scalar1=1.0,
                                        scalar2=None, op0=OP.add)
                nc.vector.tensor_tensor(out=t1[:], in0=t1[:], in1=m_t[:], op=OP.subtract)
                nc.scalar.activation(out=t1[:], in_=t1[:], func=AF.Sqrt)
                nc.vector.reciprocal(out=t1[:], in_=t1[:])
                nc.vector.tensor_tensor(out=s_own[:], in0=t1[:], in1=m_t[:], op=OP.mult)

                def emit_rsqrt_all_and_y(pool):
                    d_all = pool.tile([P, NT_GLOB], F32, name="d_all")
                    nc.sync.dma_start(out=d_all[:], in_=dg_out[:].rearrange("g p -> p g"))
                    m_a = pool.tile([P, NT_GLOB], F32, name="m_a")
                    nc.vector.tensor_scalar(out=m_a[:], in0=d_all[:], scalar1=1.0,
                                            scalar2=None, op0=OP.min)
                    t1a = pool.tile([P, NT_GLOB], F32, name="t1a")
                    nc.vector.tensor_scalar(out=t1a[:], in0=d_all[:], scalar1=1.0,
                                            scalar2=None, op0=OP.add)
                    nc.vector.tensor_tensor(out=t1a[:], in0=t1a[:], in1=m_a[:],
                                            op=OP.subtract)
                    nc.scalar.activation(out=t1a[:], in_=t1a[:], func=AF.Sqrt)
                    nc.vector.reciprocal(out=t1a[:], in_=t1a[:])
                    nc.vector.tensor_tensor(out=s_all[:], in0=t1a[:], in1=m_a[:],
                                            op=OP.mult)
                    # y = x * rsqrt(d)[node] (node-major, written to DRAM)
                    for g in range(NT_GLOB):
                        xt = pool.tile([P, C], BF16, tag="xt", bufs=3)
                        nc.sync.dma_start(out=xt[:], in_=x_full[g * P:(g + 1) * P, :])
                        yt = pool.tile([P, C], BF16, tag="yt", bufs=3)
                        nc.vector.tensor_scalar(out=yt[:], in0=xt[:],
                                                scalar1=s_all[:, g:g + 1],
                                                scalar2=None, op0=OP.mult)
                        nc.sync.dma_start(out=y_d[g * P:(g + 1) * P, :], in_=yt[:])

                # ---- QKV projections (independent of degrees; keeps PE busy) ----
                Wq_t = phA.tile([P, 2 * C], BF16)
                nc.sync.dma_start(out=Wq_t[:].rearrange("p (c n) -> p c n", c=2), in_=WqT[:].rearrange("(c p) n -> p c n", p=P))
                Wk_t = phA.tile([P, 2 * C], BF16)
                nc.sync.dma_start(out=Wk_t[:].rearrange("p (c n) -> p c n", c=2), in_=WkT[:].rearrange("(c p) n -> p c n", p=P))
                Wv_t = phA.tile([P, 2 * C], BF16)
                nc.sync.dma_start(out=Wv_t[:].rearrange("p (c n) -> p c n", c=2), in_=WvT[:].rearrange("(c p) n -> p c n", p=P))
                bq_t = phA.tile([P, 2], F32)
                nc.sync.dma_start(out=bq_t[:], in_=bq_pack[:])
                bk_t = phA.tile([P, 2], F32)
                nc.sync.dma_start(out=bk_t[:], in_=bk_pack[:])
                bv_t = phA.tile([P, C], F32)
                nc.sync.dma_start(out=bv_t[:], in_=bv_rep[:])

                # Q from xT_own
                xo = [phA.tile([P, NPC], BF16, tag=f"xo{c}", name=f"xo{c}") for c in range(2)]
                for c in range(2):
                    nc.sync.dma_start(out=xo[c][:], in_=xT_own[c * P:(c + 1) * P, :])
                for p in range(2):
                    for nb in range(NPC // 512):
                        qps = psA.tile([P, 512], F32, tag="qkps", bufs=2)
                        for c in range(2):
                            nc.tensor.matmul(
                                out=qps[:],
                                lhsT=Wq_t[:, c * C + p * P: c * C + (p + 1) * P],
                                rhs=xo[c][:, nb * 512:(nb + 1) * 512],
                                start=(c == 0), stop=(c == 1))
                        nc.vector.tensor_scalar(
                            out=QTp[p][:, nb * 512:(nb + 1) * 512], in0=qps[:],
                            scalar1=bq_t[:, p:p + 1], scalar2=None, op0=OP.add)

                # K and V in slabs of 1024 nodes
                SLAB = 1024
                for s in range(N // SLAB):
                    xts = [phA.tile([P, SLAB], BF16, tag=f"xts{c}", bufs=2, name=f"xts{c}_{s}") for c in range(2)]
                    for c in range(2):
                        nc.sync.dma_start(out=xts[c][:],
                                          in_=xT[c * P:(c + 1) * P, s * SLAB:(s + 1) * SLAB])
                    for p in range(2):
                        for nb in range(SLAB // 512):
                            kps = psA.tile([P, 512], F32, tag="qkps", bufs=2)
                            for c in range(2):
                                nc.tensor.matmul(
                                    out=kps[:],
                                    lhsT=Wk_t[:, c * C + p * P: c * C + (p + 1) * P],
                                    rhs=xts[c][:, nb * 512:(nb + 1) * 512],
                                    start=(c == 0), stop=(c == 1))
                            nc.vector.tensor_scalar(
                                out=KTp[p][:, s * SLAB + nb * 512: s * SLAB + (nb + 1) * 512],
                                in0=kps[:], scalar1=bk_t[:, p:p + 1], scalar2=None,
                                op0=OP.add)
                    for ntl in range(SLAB // P):
                        g = s * (SLAB // P) + ntl
                        vps = psA.tile([P, C], F32, tag="vps", bufs=2)
                        for c in range(2):
                            nc.tensor.matmul(
                                out=vps[:],
                                lhsT=xts[c][:, ntl * P:(ntl + 1) * P],
                                rhs=Wv_t[:, c * C:(c + 1) * C],
                                start=(c == 0), stop=(c == 1))
                        nc.vector.tensor_tensor(
                            out=V4[:, g, :, 0:DH],
                            in0=vps[:].rearrange("p (h d) -> p h d", d=DH),
                            in1=bv_t[:].rearrange("p (h d) -> p h d", d=DH),
                            op=OP.add)

                # collective has been running since before QKV; finish the
                # degree math and materialize the y table.
                emit_rsqrt_all_and_y(phA)

            # == region B: attention + GCN scatter ==
            with tc.tile_pool(name="phB", bufs=1) as phB, \
                 tc.tile_pool(name="psB", bufs=1, space="PSUM") as psB:

                # GCN scatter: single-column indirect gathers (DGE prep is
                # ~1us each on the gpsimd queue -- the pacing constraint), with
                # one-hot matmuls trailing the gathers by a prefetch margin.
                n_gath = NT_LOC * TPT
                yg_tiles = {}
                g_emitted = 0
                mm_emitted = 0
                hips_cur = {}
                ohg_b = {}

                def emit_gathers(upto):
                    nonlocal g_emitted
                    while g_emitted < min(upto, n_gath):
                        j = g_emitted
                        yg = phB.tile([P, C], BF16, tag="yg", bufs=40,
                                      name=f"yg{j}")
                        nc.gpsimd.indirect_dma_start(
                            out=yg[:], out_offset=None, in_=y_d[:],
                            in_offset=bass.IndirectOffsetOnAxis(
                                ap=row_t[:, j:j + 1], axis=0))
                        yg_tiles[j] = yg
                        g_emitted += 1

                def emit_mms(upto):
                    nonlocal mm_emitted
                    while mm_emitted < min(upto, n_gath):
                        j = mm_emitted
                        t, i = j // TPT, j % TPT
                        yg = yg_tiles.pop(j)
                        if i == 0:
                            hips_cur[t] = psB.tile([P, C], F32, tag="hips", bufs=2,
                                                   name=f"hips{t}")
                        if j % GOH == 0:
                            ohg_b[j // GOH] = phB.tile([P, GOH * P], BF16,
                                                       tag="oh2", bufs=8,
                                                       name=f"ohgb{j}")
                            nc.gpsimd.dma_start(out=ohg_b[j // GOH][:],
                                                in_=oh_in[j // GOH, :, :])
                        oh = ohg_b[j // GOH][:, (j % GOH) * P:(j % GOH + 1) * P]
                        nc.tensor.matmul(out=hips_cur[t][:], lhsT=oh, rhs=yg[:],
                                         start=(i == 0), stop=(i == TPT - 1))
                        if i == TPT - 1:
                            nc.vector.tensor_scalar(out=hi_sb[t][:], in0=hips_cur[t][:],
                                                    scalar1=s_own[:, t:t + 1],
                                                    scalar2=None, op0=OP.mult)
                        mm_emitted += 1

                # attention: scores -> exp -> O^T accumulation (2 steps behind)
                G_T0, G_T1 = 0, 224      # gather emission window (steps)
                M_T0, M_T1 = 20, 244     # scatter-matmul emission window
                step = 0

                def maybe_scatter():
                    if step >= G_T0:
                        emit_gathers(1 + n_gath * (step - G_T0) // (G_T1 - G_T0))
                    if step >= M_T0:
                        emit_mms(1 + n_gath * (step - M_T0) // (M_T1 - M_T0))

                et_t = {}

                def emit_scores(h, kt):
                    p, hh = h // 2, h % 2
                    po = hh * DH
                    sps = psB.tile([P, NPC], F32, tag="sps", bufs=2)
                    for qh in range(2):
                        nc.tensor.matmul(
                            out=sps[:, qh * 512:(qh + 1) * 512],
                            lhsT=KTp[p][po:po + DH, kt * P:(kt + 1) * P],
                            rhs=QTp[p][po:po + DH, qh * 512:(qh + 1) * 512],
                            start=True, stop=True)
                    et = phB.tile([P, NPC], BF16, tag="expT", bufs=3)
                    nc.scalar.activation(out=et[:], in_=sps[:], func=AF.Exp,
                                         bias=expb_col[:, 0:1], scale=1.0 / np.sqrt(DH))
                    et_t[(h, kt)] = et

                def emit_attnv(h, kt, OpsT):
                    et = et_t.pop((h, kt))
                    for qh in range(2):
                        nc.tensor.matmul(
                            out=OpsT[:, qh * 512:(qh + 1) * 512],
                            lhsT=V4[:, kt, h, :],
                            rhs=et[:, qh * 512:(qh + 1) * 512],
                            start=(kt == 0), stop=(kt == NT_GLOB - 1))

                OpsT_h = {}
                pend = []            # (h, kt) with scores emitted, attnV pending
                for h in range(H):
                    OpsT_h[h] = psB.tile([DH + 1, NPC], F32, tag="OpsT", bufs=1,
                                         name=f"OpsT{h}")
                    for kt in range(NT_GLOB):
                        emit_scores(h, kt)
                        pend.append((h, kt))
                        if len(pend) > 2:
                            ph_, pk_ = pend.pop(0)
                            emit_attnv(ph_, pk_, OpsT_h[ph_])
                        step += 1
                        maybe_scatter()
                    # finish this head's attnV before normalization
                    while pend:
                        ph_, pk_ = pend.pop(0)
                        emit_attnv(ph_, pk_, OpsT_h[ph_])
                    # ---- normalize O^T rows by the denominator row ----
                    # (den broadcast across partitions goes through DRAM: SBUF
                    # sources cannot have a zero partition step, DRAM can.)
                    OpsT = OpsT_h.pop(h)
                    Osb = phB.tile([DH + 1, NPC], F32, tag="Osb", bufs=2)
                    nc.vector.tensor_copy(out=Osb[:], in_=OpsT[:])
                    nc.sync.dma_start(out=den_dr[:], in_=Osb[DH:DH + 1, :])
                    bcast = phB.tile([DH, NPC], F32, tag="bcast", bufs=2)
                    nc.sync.dma_start(out=bcast[:], in_=den_dr[:].to_broadcast((DH, NPC)))
                    nc.vector.reciprocal(out=bcast[:], in_=bcast[:])
                    nc.vector.tensor_tensor(out=OTh[h][:], in0=Osb[0:DH, :],
                                            in1=bcast[:], op=OP.mult)
                emit_gathers(n_gath)
                emit_mms(n_gath)

            # ========= region C: out_proj, LN, combine, fc =========
            with tc.tile_pool(name="phC", bufs=1) as phC, \
                 tc.tile_pool(name="psC", bufs=1, space="PSUM") as psC:
                Wop_t = phC.tile([DH, H * C], BF16)
                nc.sync.dma_start(out=Wop_t[:], in_=Wop4[:])
                Wl_t = phC.tile([P, 2 * C], BF16)
                nc.sync.dma_start(out=Wl_t[:].rearrange("p (c n) -> p c n", c=2), in_=Wl[:].rearrange("(c p) n -> p c n", p=P))
                fc_t = phC.tile([P, 2 * OUTC], BF16)
                nc.sync.dma_start(out=fc_t[:].rearrange("p (c n) -> p c n", c=2), in_=fcT[:].rearrange("(c p) n -> p c n", p=P))
                opb_t = phC.tile([P, C], F32)
                nc.sync.dma_start(out=opb_t[:], in_=opb_rep[:])
                g_t = phC.tile([P, C], F32)
                nc.sync.dma_start(out=g_t[:], in_=g_rep[:])
                b_t = phC.tile([P, C], F32)
                nc.sync.dma_start(out=b_t[:], in_=b_rep[:])
                fcb_t = phC.tile([P, OUTC], F32)
                nc.sync.dma_start(out=fcb_t[:], in_=fcb_rep[:])

                def transpose_2chunks(src_ap, tag):
                    dst = phC.tile([P, C], BF16, tag=tag, bufs=2)
                    for c in range(2):
                        tp = psC.tile([P, P], BF16, tag="tp", bufs=2)
                        nc.tensor.transpose(out=tp[:], in_=src_ap[:, c * P:(c + 1) * P],
                                            identity=identb_t[:])
                        nc.vector.tensor_copy(out=dst[:, c * P:(c + 1) * P], in_=tp[:])
                    return dst

                for qt in range(NT_LOC):
                    # ---- global path: out_proj (per-head, 64-contract) + LN ----
                    aps = psC.tile([P, C], F32, tag="aps", bufs=2)
                    for h in range(H):
                        nc.tensor.matmul(out=aps[:],
                                         lhsT=OTh[h][:, qt * P:(qt + 1) * P],
                                         rhs=Wop_t[:, h * C:(h + 1) * C],
                                         start=(h == 0), stop=(h == H - 1))
                    v_t = phC.tile([P, C], F32, tag="vt", bufs=2)
                    nc.vector.tensor_tensor(out=v_t[:], in0=aps[:], in1=opb_t[:], op=OP.add)
                    xo_t = phC.tile([P, C], F32, tag="xot", bufs=2)
                    nc.sync.dma_start(out=xo_t[:], in_=x_own[qt * P:(qt + 1) * P, :])
                    nc.vector.tensor_tensor(out=v_t[:], in0=v_t[:], in1=xo_t[:], op=OP.add)
                    msum = phC.tile([P, 1], F32, tag="msum", bufs=2)
                    nc.vector.reduce_sum(out=msum[:], in_=v_t[:], axis=AX.X)
                    mean = phC.tile([P, 1], F32, tag="mean", bufs=2)
                    nc.vector.tensor_scalar(out=mean[:], in0=msum[:], scalar1=1.0 / C,
                                            scalar2=None, op0=OP.mult)
                    nc.vector.tensor_scalar(out=v_t[:], in0=v_t[:], scalar1=mean[:, 0:1],
                                            scalar2=None, op0=OP.subtract)
                    sq = phC.tile([P, C], F32, tag="sq", bufs=2)
                    nc.vector.tensor_tensor(out=sq[:], in0=v_t[:], in1=v_t[:], op=OP.mult)
                    ssum = phC.tile([P, 1], F32, tag="ssum", bufs=2)
                    nc.vector.reduce_sum(out=ssum[:], in_=sq[:], axis=AX.X)
                    sstd = phC.tile([P, 1], F32, tag="sstd", bufs=2)
                    nc.scalar.activation(out=sstd[:], in_=ssum[:], func=AF.Sqrt,
                                         bias=eps_col[:, 0:1], scale=1.0 / C)
                    rstd = phC.tile([P, 1], F32, tag="rstd", bufs=2)
                    nc.vector.reciprocal(out=rstd[:], in_=sstd[:])
                    nc.vector.tensor_scalar(out=v_t[:], in0=v_t[:], scalar1=rstd[:, 0:1],
                                            scalar2=None, op0=OP.mult)
                    nc.vector.tensor_tensor(out=v_t[:], in0=v_t[:], in1=g_t[:], op=OP.mult)
                    nc.vector.tensor_tensor(out=v_t[:], in0=v_t[:], in1=b_t[:], op=OP.add)

                    # ---- local path: local_embed = hi @ W_local ----
                    hiT = transpose_2chunks(hi_sb[qt][:], "hiT")
                    lps = psC.tile([P, C], F32, tag="lps", bufs=2)
                    for c in range(2):
                        nc.tensor.matmul(out=lps[:], lhsT=hiT[:, c * P:(c + 1) * P],
                                         rhs=Wl_t[:, c * C:(c + 1) * C],
                                         start=(c == 0), stop=(c == 1))
                    # combined = global + w * (local - global)
                    comb = phC.tile([P, C], F32, tag="comb", bufs=2)
                    nc.vector.tensor_tensor(out=comb[:], in0=lps[:], in1=v_t[:],
                                            op=OP.subtract)
                    nc.vector.tensor_scalar(out=comb[:], in0=comb[:], scalar1=w_col[:, 0:1],
                                            scalar2=None, op0=OP.mult)
                    nc.vector.tensor_tensor(out=comb[:], in0=comb[:], in1=v_t[:], op=OP.add)

                    # ---- fc ----
                    comb_b = phC.tile([P, C], BF16, tag="combb", bufs=2)
                    nc.vector.tensor_copy(out=comb_b[:], in_=comb[:])
                    cT = transpose_2chunks(comb_b[:], "cT")
                    fps = psC.tile([P, OUTC], F32, tag="fps", bufs=2)
                    for c in range(2):
                        nc.tensor.matmul(out=fps[:], lhsT=cT[:, c * P:(c + 1) * P],
                                         rhs=fc_t[:, c * OUTC:(c + 1) * OUTC],
                                         start=(c == 0), stop=(c == 1))
                    o_t = phC.tile([P, OUTC], F32, tag="ot", bufs=2)
                    nc.vector.tensor_tensor(out=o_t[:], in0=fps[:], in1=fcb_t[:], op=OP.add)
                    nc.sync.dma_start(out=out[qt * P:(qt + 1) * P, :], in_=o_t[:])
    nc.finalize()
    return nc


def _prep_edges(adj):
    """Bucket edges by destination node-tile; pad segments to a common length.

    Returns per-core (oh[TE, P, P] bf16 one-hot tiles, idx16[P, S*NT_LOC//16]
    wrapped gather indices) and TPT (edge tiles per segment). Layout only:
    one-hot encoding of the within-tile destination and the dma_gather index
    wrap are both re-encodings of the adjacency list.
    """
    BF = ml_dtypes.bfloat16
    row = np.asarray(adj[0], dtype=np.int64)
    col = np.asarray(adj[1], dtype=np.int64)
    tid = col // P
    order = np.argsort(tid, kind='stable')
    row_s, col_s = row[order], col[order]
    counts = np.bincount(tid, minlength=NT_GLOB)
    S = int(np.ceil(max(counts.max(), 1) / P) * P)
    TPT = S // P
    col_pad = np.full((NT_GLOB, S), -1, dtype=np.int32)
    row_pad = np.zeros((NT_GLOB, S), dtype=np.int32)
    start = 0
    for g in range(NT_GLOB):
        cnt = int(counts[g])
        col_pad[g, :cnt] = (col_s[start:start + cnt] - g * P).astype(np.int32)
        row_pad[g, :cnt] = row_s[start:start + cnt].astype(np.int32)
        start += cnt
    col_pad = col_pad.reshape(NT_GLOB, TPT, P)      # [g, i, e] within-tile dest
    row_pad = row_pad.reshape(NT_GLOB, TPT, P)      # [g, i, e] source row
    GOH = 4
    TE = NT_LOC * TPT
    G_CNT = (TE + GOH - 1) // GOH
    per_core = []
    for k in range(NCORES):
        cp = col_pad[NT_LOC * k:NT_LOC * (k + 1)].reshape(TE, P)
        oh = (cp[:, :, None] == np.arange(P, dtype=np.int32)[None, None, :])
        oh = oh.astype(BF)                          # [TE, e, d]
        if G_CNT * GOH > TE:
            oh = np.concatenate([oh, np.zeros((G_CNT * GOH - TE, P, P), BF)], 0)
        # pack GOH tiles side by side in the free dim: [G_CNT, e, GOH*d]
        ohg = np.ascontiguousarray(
            oh.reshape(G_CNT, GOH, P, P).transpose(0, 2, 1, 3).reshape(G_CNT, P, GOH * P))
        ri = row_pad[NT_LOC * k:NT_LOC * (k + 1)].reshape(TE, P).T
        per_core.append((ohg, np.ascontiguousarray(ri.astype(np.int32))))
    return per_core, TPT


def kernel(x, adj, weight_local, in_proj_w, in_proj_b, out_proj_w, out_proj_b,
           ln_g, ln_b, alpha, fc_w, fc_b):
    global LAST_RESULTS
    BF = ml_dtypes.bfloat16
    x = np.ascontiguousarray(np.asarray(x, dtype=np.float32))
    per_core_edges, TPT = _prep_edges(np.asarray(adj))

    xb = x.astype(BF)
    xTb = np.ascontiguousarray(xb.T)
    WopT = np.asarray(out_proj_w).T.astype(np.float32)  # [C_in, C_out]
    common = dict(
        xT=xTb,
        x_full=xb,
        WqT=np.ascontiguousarray(np.asarray(in_proj_w)[0:C].T.astype(BF)),
        WkT=np.ascontiguousarray(np.asarray(in_proj_w)[C:2 * C].T.astype(BF)),
        WvT=np.ascontiguousarray(np.asarray(in_proj_w)[2 * C:3 * C].T.astype(BF)),
        Wop4=np.ascontiguousarray(
            WopT.reshape(H, DH, C).transpose(1, 0, 2).reshape(DH, H * C).astype(BF)),
        Wl=np.ascontiguousarray(np.asarray(weight_local).astype(BF)),
        fcT=np.ascontiguousarray(np.asarray(fc_w).T.astype(BF)),
        bq_pack=np.ascontiguousarray(np.asarray(in_proj_b)[0:C].astype(np.float32).reshape(2, P).T),
        bk_pack=np.ascontiguousarray(np.asarray(in_proj_b)[C:2 * C].astype(np.float32).reshape(2, P).T),
        bv_rep=np.tile(np.asarray(in_proj_b)[2 * C:3 * C].astype(np.float32), (P, 1)),
        opb_rep=np.tile(np.asarray(out_proj_b, dtype=np.float32), (P, 1)),
        g_rep=np.tile(np.asarray(ln_g, dtype=np.float32), (P, 1)),
        b_rep=np.tile(np.asarray(ln_b, dtype=np.float32), (P, 1)),
        fcb_rep=np.tile(np.asarray(fc_b, dtype=np.float32), (P, 1)),
        alpha11=np.asarray(alpha, dtype=np.float32).reshape(1, 1),
        identb_in=np.eye(P, dtype=np.float32).astype(BF),
        ones_col_in=np.ones((P, 1), np.float32).astype(BF),
        ones_row_in=np.ones((1, P), dtype=np.float32),
    )
    in_maps = []
    for k in range(NCORES):
        ohg, ri = per_core_edges[k]
        m = dict(common)
        m['xT_own'] = np.ascontiguousarray(xTb[:, k * NPC:(k + 1) * NPC])
        m['x_own'] = np.ascontiguousarray(x[k * NPC:(k + 1) * NPC, :])
        m['oh_in'] = ohg
        m['row_idx'] = ri
        in_maps.append(m)

    nc = _build(TPT)
    res = run_bass_kernel_spmd(nc, in_maps, core_ids=list(range(NCORES)))
    LAST_RESULTS = res
    return np.concatenate([res.results[k]['out'] for k in range(NCORES)], axis=0)


# revision 18
# speedup vs baseline: 1.2033x; 1.2033x over previous
"""Trainium2 Bass kernel for LocalGlobalEnvEncoder (GCN + MHA fusion).

Sharding: nodes are split across the 8 cores (1024 dest nodes / queries each).
 - GCN: edges bucketed by destination node-tile on host (layout only); degrees
   computed on-device via one-hot matmuls, exchanged with an AllGather; messages
   gathered from a device-materialized y = x * rsqrt(d) table via batched
   indirect DMA and scatter-added with one-hot matmuls on the PE.
 - MHA: query-sharded attention, K/V computed redundantly per core, scores kept
   transposed ([key, query]); attn@V accumulates O^T per head with an appended
   ones-column in V providing softmax denominators.
Region A: degrees + QKV projections (PE busy while indices load).
Region B: AllGather + y table + attention + interleaved GCN scatter.
Region C: out_proj + LN + combine + fc.
All heavy matmuls run in bf16 (fp32 PSUM accumulation); normalization fp32.
"""
import sys
sys.path.insert(0, '/opt/trn_rl_repo')
import numpy as np
import ml_dtypes
import concourse.bass as bass
import concourse.tile as tile
from concourse import bacc, mybir, library_config
from concourse.bass_utils import run_bass_kernel_spmd

F32 = mybir.dt.float32
BF16 = mybir.dt.bfloat16
I32 = mybir.dt.int32
AF = mybir.ActivationFunctionType
OP = mybir.AluOpType
AX = mybir.AxisListType

N, E, C, OUTC, H, DH = 8192, 262144, 256, 256, 4, 64
NCORES = 8
NPC = N // NCORES          # nodes per core = 1024
P = 128
NT_LOC = NPC // P          # node tiles per core = 8
NT_GLOB = N // P           # global node tiles = 64
EXP_BIAS = -12.0           # uniform shift inside softmax exp; cancels in the ratio
GATHER_GRP = 8             # edge tiles per indirect-DMA gather (1024 descriptors)

LAST_RESULTS = None        # stashed BassKernelResults for test harness introspection


def _build(TPT):
    """Build the single SPMD Bass program. TPT = edge tiles per node-tile segment."""
    nc = bacc.Bacc('TRN2', target_bir_lowering=False, debug=False, num_devices=NCORES)
    TE = NT_LOC * TPT  # total edge tiles per core

    # ---- I/O ----
    xT = nc.dram_tensor("xT", [C, N], BF16, kind="ExternalInput")
    xT_own = nc.dram_tensor("xT_own", [C, NPC], BF16, kind="ExternalInput")
    x_full = nc.dram_tensor("x_full", [N, C], BF16, kind="ExternalInput")
    x_own = nc.dram_tensor("x_own", [NPC, C], F32, kind="ExternalInput")
    WqT = nc.dram_tensor("WqT", [C, C], BF16, kind="ExternalInput")
    WkT = nc.dram_tensor("WkT", [C, C], BF16, kind="ExternalInput")
    WvT = nc.dram_tensor("WvT", [C, C], BF16, kind="ExternalInput")
    Wop4 = nc.dram_tensor("Wop4", [DH, H * C], BF16, kind="ExternalInput")
    Wl = nc.dram_tensor("Wl", [C, C], BF16, kind="ExternalInput")
    fcT = nc.dram_tensor("fcT", [C, OUTC], BF16, kind="ExternalInput")
    bq_pack = nc.dram_tensor("bq_pack", [P, 2], F32, kind="ExternalInput")
    bk_pack = nc.dram_tensor("bk_pack", [P, 2], F32, kind="ExternalInput")
    bv_rep = nc.dram_tensor("bv_rep", [P, C], F32, kind="ExternalInput")
    opb_rep = nc.dram_tensor("opb_rep", [P, C], F32, kind="ExternalInput")
    g_rep = nc.dram_tensor("g_rep", [P, C], F32, kind="ExternalInput")
    b_rep = nc.dram_tensor("b_rep", [P, C], F32, kind="ExternalInput")
    fcb_rep = nc.dram_tensor("fcb_rep", [P, OUTC], F32, kind="ExternalInput")
    alpha11 = nc.dram_tensor("alpha11", [1, 1], F32, kind="ExternalInput")
    identb_in = nc.dram_tensor("identb_in", [P, P], BF16, kind="ExternalInput")
    ones_col_in = nc.dram_tensor("ones_col_in", [P, 1], BF16, kind="ExternalInput")
    ones_row_in = nc.dram_tensor("ones_row_in", [1, P], F32, kind="ExternalInput")
    GOH = 8                    # one-hot tiles packed per DMA
    G_CNT = (TE + GOH - 1) // GOH
    oh_in = nc.dram_tensor("oh_in", [G_CNT, P, GOH * P], BF16, kind="ExternalInput")
    row_idx = nc.dram_tensor("row_idx", [P, TE], I32, kind="ExternalInput")

    out = nc.dram_tensor("out", [NPC, OUTC], F32, kind="ExternalOutput")

    with tile.TileContext(nc) as tc:
        with tc.tile_pool(name="const", bufs=1) as const, \
             tc.tile_pool(name="big", bufs=1) as big, \
             tc.tile_pool(name="dram", bufs=1, space="DRAM") as dram:

            # ---- persistent constants ----
            identb_t = const.tile([P, P], BF16)
            nc.sync.dma_start(out=identb_t[:], in_=identb_in[:])
            ones_col_t = const.tile([P, 1], BF16)
            nc.sync.dma_start(out=ones_col_t[:], in_=ones_col_in[:])
            ones_row_t = const.tile([1, P], F32)
            nc.sync.dma_start(out=ones_row_t[:], in_=ones_row_in[:])
            row_t = const.tile([P, TE], I32)
            nc.sync.dma_start(out=row_t[:], in_=row_idx[:])
            expb_col = const.tile([P, 1], F32)
            nc.vector.memset(expb_col[:], EXP_BIAS)
            eps_col = const.tile([P, 1], F32)
            nc.vector.memset(eps_col[:], 1e-5)

            d_loc = const.tile([P, NT_LOC], F32)
            s_own = const.tile([P, NT_LOC], F32)
            s_all = const.tile([P, NT_GLOB], F32)
            w_col = const.tile([P, 1], F32)
            dg_in = dram.tile([NT_LOC, P], F32)
            dg_out = dram.tile([NT_GLOB, P], F32)
            y_d = dram.tile([N, C], BF16)
            den_dr = dram.tile([1, NPC], F32)

            # persistent SBUF state across regions
            KTp = [big.tile([P, N], BF16, name=f"KT{p}") for p in range(2)]
            QTp = [big.tile([P, NPC], BF16, name=f"QT{p}") for p in range(2)]
            Vt = big.tile([P, NT_GLOB * H * (DH + 1)], BF16, name="Vt")
            V4 = Vt[:].rearrange("p (k h d) -> p k h d", h=H, d=DH + 1)
            OTh = [big.tile([DH, NPC], BF16, name=f"OT{h}") for h in range(H)]
            hi_sb = [big.tile([P, C], BF16, name=f"hi{i}") for i in range(NT_LOC)]

            nc.vector.memset(V4[:, :, :, DH:DH + 1], 1.0)  # ones col -> denominators

            # ============ region A: degrees + sigmoid(alpha) + QKV ============
            with tc.tile_pool(name="phA", bufs=1) as phA, \
                 tc.tile_pool(name="psA", bufs=1, space="PSUM") as psA:
                # sigmoid(alpha) replicated to a [128,1] column
                al_t = phA.tile([1, 1], F32)
                nc.sync.dma_start(out=al_t[:], in_=alpha11[:])
                wsig = phA.tile([1, 1], F32)
                nc.scalar.activation(out=wsig[:], in_=al_t[:], func=AF.Sigmoid)
                wrep_ps = psA.tile([P, 1], F32, tag="wrep")
                nc.tensor.matmul(out=wrep_ps[:], lhsT=ones_row_t[:], rhs=wsig[:],
                                 start=True, stop=True)
                nc.vector.tensor_copy(out=w_col[:], in_=wrep_ps[:])

                # degrees of own nodes via one-hot matmuls against a ones column
                ohg_a = {}
                for t in range(NT_LOC):
                    dps = psA.tile([P, 1], F32, tag="dps", bufs=2)
                    for i in range(TPT):
                        j = t * TPT + i
                        if j % GOH == 0:
                            ohg_a[j // GOH] = phA.tile([P, GOH * P], BF16, tag="oh", bufs=12, name=f"ohga{j}")
                            nc.sync.dma_start(out=ohg_a[j // GOH][:], in_=oh_in[j // GOH, :, :])
                        oh = ohg_a[j // GOH][:, (j % GOH) * P:(j % GOH + 1) * P]
                        nc.tensor.matmul(out=dps[:], lhsT=oh, rhs=ones_col_t[:],
                                         start=(i == 0), stop=(i == TPT - 1))
                    nc.vector.tensor_copy(out=d_loc[:, t:t + 1], in_=dps[:])
                    nc.sync.dma_start(out=dg_in[t, :, None], in_=d_loc[:, t:t + 1])

                # launch the degree exchange early; it overlaps the QKV work
                nc.gpsimd.collective_compute(
                    "AllGather", OP.bypass,
                    replica_groups=[list(range(NCORES))],
                    ins=[dg_in[:].opt()], outs=[dg_out[:].opt()])

                # rsqrt for own nodes (s_own); s_all comes after the AllGather
                m_t = phA.tile([P, NT_LOC], F32, tag="mo")
                nc.vector.tensor_scalar(out=m_t[:], in0=d_loc[:], scalar1=1.0,
                                        scalar2=None, op0=OP.min)
                t1 = phA.tile([P, NT_LOC], F32, tag="t1o")
                nc.vector.tensor_scalar(out=t1[:], in0=d_loc[:], scalar1=1.0,
                                        scalar2=None, op0=OP.add)
                nc.vector.tensor_tensor(out=t1[:], in0=t1[:], in1=m_t[:], op=OP.subtract)
                nc.scalar.activation(out=t1[:], in_=t1[:], func=AF.Sqrt)
                nc.vector.reciprocal(out=t1[:], in_=t1[:])
                nc.vector.tensor_tensor(out=s_own[:], in0=t1[:], in1=m_t[:], op=OP.mult)

                def emit_rsqrt_all_and_y(pool):
                    d_all = pool.tile([P, NT_GLOB], F32, name="d_all")
                    nc.sync.dma_start(out=d_all[:], in_=dg_out[:].rearrange("g p -> p g"))
                    m_a = pool.tile([P, NT_GLOB], F32, name="m_a")
                    nc.vector.tensor_scalar(out=m_a[:], in0=d_all[:], scalar1=1.0,
                                            scalar2=None, op0=OP.min)
                    t1a = pool.tile([P, NT_GLOB], F32, name="t1a")
                    nc.vector.tensor_scalar(out=t1a[:], in0=d_all[:], scalar1=1.0,
                                            scalar2=None, op0=OP.add)
                    nc.vector.tensor_tensor(out=t1a[:], in0=t1a[:], in1=m_a[:],
                                            op=OP.subtract)
                    nc.scalar.activation(out=t1a[:], in_=t1a[:], func=AF.Sqrt)
                    nc.vector.reciprocal(out=t1a[:], in_=t1a[:])
                    nc.vector.tensor_tensor(out=s_all[:], in0=t1a[:], in1=m_a[:],
                                            op=OP.mult)
                    # y = x * rsqrt(d)[node] (node-major, written to DRAM),
                    # 4 node-tiles per DMA to keep queue occupancy low
                    GY = 4
                    for g4 in range(NT_GLOB // GY):
                        xt = pool.tile([P, GY, C], BF16, tag="xt", bufs=3)
                        nc.scalar.dma_start(
                            out=xt[:],
                            in_=x_full[g4 * GY * P:(g4 + 1) * GY * P, :]
                                .rearrange("(t p) c -> p t c", p=P))
                        yt = pool.tile([P, GY, C], BF16, tag="yt", bufs=3)
                        for u in range(GY):
                            nc.vector.tensor_scalar(
                                out=yt[:, u, :], in0=xt[:, u, :],
                                scalar1=s_all[:, g4 * GY + u:g4 * GY + u + 1],
                                scalar2=None, op0=OP.mult)
                        nc.sync.dma_start(
                            out=y_d[g4 * GY * P:(g4 + 1) * GY * P, :]
                                .rearrange("(t p) c -> p t c", p=P),
                            in_=yt[:])

                # ---- QKV projections (independent of degrees; keeps PE busy) ----
                Wq_t = phA.tile([P, 2 * C], BF16)
                nc.sync.dma_start(out=Wq_t[:].rearrange("p (c n) -> p c n", c=2), in_=WqT[:].rearrange("(c p) n -> p c n", p=P))
                Wk_t = phA.tile([P, 2 * C], BF16)
                nc.sync.dma_start(out=Wk_t[:].rearrange("p (c n) -> p c n", c=2), in_=WkT[:].rearrange("(c p) n -> p c n", p=P))
                Wv_t = phA.tile([P, 2 * C], BF16)
                nc.sync.dma_start(out=Wv_t[:].rearrange("p (c n) -> p c n", c=2), in_=WvT[:].rearrange("(c p) n -> p c n", p=P))
                bq_t = phA.tile([P, 2], F32)
                nc.sync.dma_start(out=bq_t[:], in_=bq_pack[:])
                bk_t = phA.tile([P, 2], F32)
                nc.sync.dma_start(out=bk_t[:], in_=bk_pack[:])
                bv_t = phA.tile([P, C], F32)
                nc.sync.dma_start(out=bv_t[:], in_=bv_rep[:])

                # Q from xT_own
                xo = [phA.tile([P, NPC], BF16, tag=f"xo{c}", name=f"xo{c}") for c in range(2)]
                for c in range(2):
                    nc.sync.dma_start(out=xo[c][:], in_=xT_own[c * P:(c + 1) * P, :])
                for p in range(2):
                    for nb in range(NPC // 512):
                        qps = psA.tile([P, 512], F32, tag="qkps", bufs=2)
                        for c in range(2):
                            nc.tensor.matmul(
                                out=qps[:],
                                lhsT=Wq_t[:, c * C + p * P: c * C + (p + 1) * P],
                                rhs=xo[c][:, nb * 512:(nb + 1) * 512],
                                start=(c == 0), stop=(c == 1))
                        nc.vector.tensor_scalar(
                            out=QTp[p][:, nb * 512:(nb + 1) * 512], in0=qps[:],
                            scalar1=bq_t[:, p:p + 1], scalar2=None, op0=OP.add)

                # K and V in slabs of 1024 nodes
                SLAB = 1024
                for s in range(N // SLAB):
                    xts = [phA.tile([P, SLAB], BF16, tag=f"xts{c}", bufs=2, name=f"xts{c}_{s}") for c in range(2)]
                    for c in range(2):
                        nc.sync.dma_start(out=xts[c][:],
                                          in_=xT[c * P:(c + 1) * P, s * SLAB:(s + 1) * SLAB])
                    for p in range(2):
                        for nb in range(SLAB // 512):
                            kps = psA.tile([P, 512], F32, tag="qkps", bufs=2)
                            for c in range(2):
                                nc.tensor.matmul(
                                    out=kps[:],
                                    lhsT=Wk_t[:, c * C + p * P: c * C + (p + 1) * P],
                                    rhs=xts[c][:, nb * 512:(nb + 1) * 512],
                                    start=(c == 0), stop=(c == 1))
                            nc.vector.tensor_scalar(
                                out=KTp[p][:, s * SLAB + nb * 512: s * SLAB + (nb + 1) * 512],
                                in0=kps[:], scalar1=bk_t[:, p:p + 1], scalar2=None,
                                op0=OP.add)
                    for ntl in range(SLAB // P):
                        g = s * (SLAB // P) + ntl
                        vps = psA.tile([P, C], F32, tag="vps", bufs=2)
                        for c in range(2):
                            nc.tensor.matmul(
                                out=vps[:],
                                lhsT=xts[c][:, ntl * P:(ntl + 1) * P],
                                rhs=Wv_t[:, c * C:(c + 1) * C],
                                start=(c == 0), stop=(c == 1))
                        nc.vector.tensor_tensor(
                            out=V4[:, g, :, 0:DH],
                            in0=vps[:].rearrange("p (h d) -> p h d", d=DH),
                            in1=bv_t[:].rearrange("p (h d) -> p h d", d=DH),
                            op=OP.add)

                # collective has been running since before QKV; finish the
                # degree math and materialize the y table.
                emit_rsqrt_all_and_y(phA)

            # == region B: attention + GCN scatter ==
            with tc.tile_pool(name="phB", bufs=1) as phB, \
                 tc.tile_pool(name="psB", bufs=1, space="PSUM") as psB:

                # GCN scatter: single-column indirect gathers (DGE prep is
                # ~1us each on the gpsimd queue -- the pacing constraint), with
                # one-hot matmuls trailing the gathers by a prefetch margin.
                n_gath = NT_LOC * TPT
                yg_tiles = {}
                g_emitted = 0
                mm_emitted = 0
                hips_cur = {}
                ohg_b = {}

                def emit_gathers(upto):
                    nonlocal g_emitted
                    while g_emitted < min(upto, n_gath):
                        j = g_emitted
                        yg = phB.tile([P, C], BF16, tag="yg", bufs=40,
                                      name=f"yg{j}")
                        nc.gpsimd.indirect_dma_start(
                            out=yg[:], out_offset=None, in_=y_d[:],
                            in_offset=bass.IndirectOffsetOnAxis(
                                ap=row_t[:, j:j + 1], axis=0))
                        yg_tiles[j] = yg
                        g_emitted += 1

                def emit_mms(upto):
                    nonlocal mm_emitted
                    while mm_emitted < min(upto, n_gath):
                        j = mm_emitted
                        t, i = j // TPT, j % TPT
                        yg = yg_tiles.pop(j)
                        if i == 0:
                            hips_cur[t] = psB.tile([P, C], F32, tag="hips", bufs=2,
                                                   name=f"hips{t}")
                        if j % GOH == 0:
                            ohg_b[j // GOH] = phB.tile([P, GOH * P], BF16,
                                                       tag="oh2", bufs=8,
                                                       name=f"ohgb{j}")
                            nc.sync.dma_start(out=ohg_b[j // GOH][:],
                                                in_=oh_in[j // GOH, :, :])
                        oh = ohg_b[j // GOH][:, (j % GOH) * P:(j % GOH + 1) * P]
                        nc.tensor.matmul(out=hips_cur[t][:], lhsT=oh, rhs=yg[:],
                                         start=(i == 0), stop=(i == TPT - 1))
                        if i == TPT - 1:
                            nc.vector.tensor_scalar(out=hi_sb[t][:], in0=hips_cur[t][:],
                                                    scalar1=s_own[:, t:t + 1],
                                                    scalar2=None, op0=OP.mult)
                        mm_emitted += 1

                # attention: scores -> exp -> O^T accumulation (2 steps behind)
                G_T0, G_T1 = 0, 232      # gather emission window (steps)
                M_T0, M_T1 = 16, 248     # scatter-matmul emission window
                step = 0

                def maybe_scatter():
                    if step >= G_T0:
                        emit_gathers(1 + n_gath * (step - G_T0) // (G_T1 - G_T0))
                    if step >= M_T0:
                        emit_mms(1 + n_gath * (step - M_T0) // (M_T1 - M_T0))

                et_t = {}

                def emit_scores(h, kt):
                    p, hh = h // 2, h % 2
                    po = hh * DH
                    sps = psB.tile([P, NPC], F32, tag="sps", bufs=2)
                    for qh in range(2):
                        nc.tensor.matmul(
                            out=sps[:, qh * 512:(qh + 1) * 512],
                            lhsT=KTp[p][po:po + DH, kt * P:(kt + 1) * P],
                            rhs=QTp[p][po:po + DH, qh * 512:(qh + 1) * 512],
                            start=True, stop=True)
                    et = phB.tile([P, NPC], BF16, tag="expT", bufs=3)
                    nc.scalar.activation(out=et[:], in_=sps[:], func=AF.Exp,
                                         bias=expb_col[:, 0:1], scale=1.0 / np.sqrt(DH))
                    et_t[(h, kt)] = et

                def emit_attnv(h, kt, OpsT):
                    et = et_t.pop((h, kt))
                    for qh in range(2):
                        nc.tensor.matmul(
                            out=OpsT[:, qh * 512:(qh + 1) * 512],
                            lhsT=V4[:, kt, h, :],
                            rhs=et[:, qh * 512:(qh + 1) * 512],
                            start=(kt == 0), stop=(kt == NT_GLOB - 1))

                OpsT_h = {}
                pend = []            # (h, kt) with scores emitted, attnV pending
                for h in range(H):
                    OpsT_h[h] = psB.tile([DH + 1, NPC], F32, tag="OpsT", bufs=1,
                                         name=f"OpsT{h}")
                    for kt in range(NT_GLOB):
                        emit_scores(h, kt)
                        pend.append((h, kt))
                        if len(pend) > 2:
                            ph_, pk_ = pend.pop(0)
                            emit_attnv(ph_, pk_, OpsT_h[ph_])
                        step += 1
                        maybe_scatter()
                    # finish this head's attnV before normalization
                    while pend:
                        ph_, pk_ = pend.pop(0)
                        emit_attnv(ph_, pk_, OpsT_h[ph_])
                    # ---- normalize O^T rows by the denominator row ----
                    # (den broadcast across partitions goes through DRAM: SBUF
                    # sources cannot have a zero partition step, DRAM can.)
                    OpsT = OpsT_h.pop(h)
                    Osb = phB.tile([DH + 1, NPC], F32, tag="Osb", bufs=2)
                    nc.vector.tensor_copy(out=Osb[:], in_=OpsT[:])
                    nc.sync.dma_start(out=den_dr[:], in_=Osb[DH:DH + 1, :])
                    bcast = phB.tile([DH, NPC], F32, tag="bcast", bufs=2)
                    nc.sync.dma_start(out=bcast[:], in_=den_dr[:].to_broadcast((DH, NPC)))
                    nc.vector.reciprocal(out=bcast[:], in_=bcast[:])
                    nc.vector.tensor_tensor(out=OTh[h][:], in0=Osb[0:DH, :],
                                            in1=bcast[:], op=OP.mult)
                emit_gathers(n_gath)
                emit_mms(n_gath)

            # ========= region C: out_proj, LN, combine, fc =========
            with tc.tile_pool(name="phC", bufs=1) as phC, \
                 tc.tile_pool(name="psC", bufs=1, space="PSUM") as psC:
                Wop_t = phC.tile([DH, H * C], BF16)
                nc.sync.dma_start(out=Wop_t[:], in_=Wop4[:])
                Wl_t = phC.tile([P, 2 * C], BF16)
                nc.sync.dma_start(out=Wl_t[:].rearrange("p (c n) -> p c n", c=2), in_=Wl[:].rearrange("(c p) n -> p c n", p=P))
                fc_t = phC.tile([P, 2 * OUTC], BF16)
                nc.sync.dma_start(out=fc_t[:].rearrange("p (c n) -> p c n", c=2), in_=fcT[:].rearrange("(c p) n -> p c n", p=P))
                opb_t = phC.tile([P, C], F32)
                nc.sync.dma_start(out=opb_t[:], in_=opb_rep[:])
                g_t = phC.tile([P, C], F32)
                nc.sync.dma_start(out=g_t[:], in_=g_rep[:])
                b_t = phC.tile([P, C], F32)
                nc.sync.dma_start(out=b_t[:], in_=b_rep[:])
                fcb_t = phC.tile([P, OUTC], F32)
                nc.sync.dma_start(out=fcb_t[:], in_=fcb_rep[:])

                def transpose_2chunks(src_ap, tag):
                    dst = phC.tile([P, C], BF16, tag=tag, bufs=2)
                    for c in range(2):
                        tp = psC.tile([P, P], BF16, tag="tp", bufs=2)
                        nc.tensor.transpose(out=tp[:], in_=src_ap[:, c * P:(c + 1) * P],
                                            identity=identb_t[:])
                        nc.vector.tensor_copy(out=dst[:, c * P:(c + 1) * P], in_=tp[:])
                    return dst

                for qt in range(NT_LOC):
                    # ---- global path: out_proj (per-head, 64-contract) + LN ----
                    aps = psC.tile([P, C], F32, tag="aps", bufs=2)
                    for h in range(H):
                        nc.tensor.matmul(out=aps[:],
                                         lhsT=OTh[h][:, qt * P:(qt + 1) * P],
                                         rhs=Wop_t[:, h * C:(h + 1) * C],
                                         start=(h == 0), stop=(h == H - 1))
                    v_t = phC.tile([P, C], F32, tag="vt", bufs=2)
                    nc.vector.tensor_tensor(out=v_t[:], in0=aps[:], in1=opb_t[:], op=OP.add)
                    xo_t = phC.tile([P, C], F32, tag="xot", bufs=2)
                    nc.sync.dma_start(out=xo_t[:], in_=x_own[qt * P:(qt + 1) * P, :])
                    nc.vector.tensor_tensor(out=v_t[:], in0=v_t[:], in1=xo_t[:], op=OP.add)
                    msum = phC.tile([P, 1], F32, tag="msum", bufs=2)
                    nc.vector.reduce_sum(out=msum[:], in_=v_t[:], axis=AX.X)
                    mean = phC.tile([P, 1], F32, tag="mean", bufs=2)
                    nc.vector.tensor_scalar(out=mean[:], in0=msum[:], scalar1=1.0 / C,
                                            scalar2=None, op0=OP.mult)
                    nc.vector.tensor_scalar(out=v_t[:], in0=v_t[:], scalar1=mean[:, 0:1],
                                            scalar2=None, op0=OP.subtract)
                    sq = phC.tile([P, C], F32, tag="sq", bufs=2)
                    nc.vector.tensor_tensor(out=sq[:], in0=v_t[:], in1=v_t[:], op=OP.mult)
                    ssum = phC.tile([P, 1], F32, tag="ssum", bufs=2)
                    nc.vector.reduce_sum(out=ssum[:], in_=sq[:], axis=AX.X)
                    sstd = phC.tile([P, 1], F32, tag="sstd", bufs=2)
                    nc.scalar.activation(out=sstd[:], in_=ssum[:], func=AF.Sqrt,
                                         bias=eps_col[:, 0:1], scale=1.0 / C)
                    rstd = phC.tile([P, 1], F32, tag="rstd", bufs=2)
                    nc.vector.reciprocal(out=rstd[:], in_=sstd[:])
                    nc.vector.tensor_scalar(out=v_t[:], in0=v_t[:], scalar1=rstd[:, 0:1],
                                            scalar2=None, op0=OP.mult)
                    nc.vector.tensor_tensor(out=v_t[:], in0=v_t[:], in1=g_t[:], op=OP.mult)
                    nc.vector.tensor_tensor(out=v_t[:], in0=v_t[:], in1=b_t[:], op=OP.add)

                    # ---- local path: local_embed = hi @ W_local ----
                    hiT = transpose_2chunks(hi_sb[qt][:], "hiT")
                    lps = psC.tile([P, C], F32, tag="lps", bufs=2)
                    for c in range(2):
                        nc.tensor.matmul(out=lps[:], lhsT=hiT[:, c * P:(c + 1) * P],
                                         rhs=Wl_t[:, c * C:(c + 1) * C],
                                         start=(c == 0), stop=(c == 1))
                    # combined = global + w * (local - global)
                    comb = phC.tile([P, C], F32, tag="comb", bufs=2)
                    nc.vector.tensor_tensor(out=comb[:], in0=lps[:], in1=v_t[:],
                                            op=OP.subtract)
                    nc.vector.tensor_scalar(out=comb[:], in0=comb[:], scalar1=w_col[:, 0:1],
                                            scalar2=None, op0=OP.mult)
                    nc.vector.tensor_tensor(out=comb[:], in0=comb[:], in1=v_t[:], op=OP.add)

                    # ---- fc ----
                    comb_b = phC.tile([P, C], BF16, tag="combb", bufs=2)
                    nc.vector.tensor_copy(out=comb_b[:], in_=comb[:])
                    cT = transpose_2chunks(comb_b[:], "cT")
                    fps = psC.tile([P, OUTC], F32, tag="fps", bufs=2)
                    for c in range(2):
                        nc.tensor.matmul(out=fps[:], lhsT=cT[:, c * P:(c + 1) * P],
                                         rhs=fc_t[:, c * OUTC:(c + 1) * OUTC],
                                         start=(c == 0), stop=(c == 1))
                    o_t = phC.tile([P, OUTC], F32, tag="ot", bufs=2)
                    nc.vector.tensor_tensor(out=o_t[:], in0=fps[:], in1=fcb_t[:], op=OP.add)
                    nc.sync.dma_start(out=out[qt * P:(qt + 1) * P, :], in_=o_t[:])
    nc.finalize()
    return nc


def _prep_edges(adj):
    """Bucket edges by destination node-tile; pad segments to a common length.

    Returns per-core (oh[TE, P, P] bf16 one-hot tiles, idx16[P, S*NT_LOC//16]
    wrapped gather indices) and TPT (edge tiles per segment). Layout only:
    one-hot encoding of the within-tile destination and the dma_gather index
    wrap are both re-encodings of the adjacency list.
    """
    BF = ml_dtypes.bfloat16
    row = np.asarray(adj[0], dtype=np.int64)
    col = np.asarray(adj[1], dtype=np.int64)
    tid = col // P
    order = np.argsort(tid, kind='stable')
    row_s, col_s = row[order], col[order]
    counts = np.bincount(tid, minlength=NT_GLOB)
    S = int(np.ceil(max(counts.max(), 1) / P) * P)
    TPT = S // P
    col_pad = np.full((NT_GLOB, S), -1, dtype=np.int32)
    row_pad = np.zeros((NT_GLOB, S), dtype=np.int32)
    start = 0
    for g in range(NT_GLOB):
        cnt = int(counts[g])
        col_pad[g, :cnt] = (col_s[start:start + cnt] - g * P).astype(np.int32)
        row_pad[g, :cnt] = row_s[start:start + cnt].astype(np.int32)
        start += cnt
    col_pad = col_pad.reshape(NT_GLOB, TPT, P)      # [g, i, e] within-tile dest
    row_pad = row_pad.reshape(NT_GLOB, TPT, P)      # [g, i, e] source row
    GOH = 8
    TE = NT_LOC * TPT
    G_CNT = (TE + GOH - 1) // GOH
    per_core = []
    for k in range(NCORES):
        cp = col_pad[NT_LOC * k:NT_LOC * (k + 1)].reshape(TE, P)
        oh = (cp[:, :, None] == np.arange(P, dtype=np.int32)[None, None, :])
        oh = oh.astype(BF)                          # [TE, e, d]
        if G_CNT * GOH > TE:
            oh = np.concatenate([oh, np.zeros((G_CNT * GOH - TE, P, P), BF)], 0)
        # pack GOH tiles side by side in the free dim: [G_CNT, e, GOH*d]
        ohg = np.ascontiguousarray(
            oh.reshape(G_CNT, GOH, P, P).transpose(0, 2, 1, 3).reshape(G_CNT, P, GOH * P))
        ri = row_pad[NT_LOC * k:NT_LOC * (k + 1)].reshape(TE, P).T
        per_core.append((ohg, np.ascontiguousarray(ri.astype(np.int32))))
    return per_core, TPT


def kernel(x, adj, weight_local, in_proj_w, in_proj_b, out_proj_w, out_proj_b,
           ln_g, ln_b, alpha, fc_w, fc_b):
    global LAST_RESULTS
    BF = ml_dtypes.bfloat16
    x = np.ascontiguousarray(np.asarray(x, dtype=np.float32))
    per_core_edges, TPT = _prep_edges(np.asarray(adj))

    xb = x.astype(BF)
    xTb = np.ascontiguousarray(xb.T)
    WopT = np.asarray(out_proj_w).T.astype(np.float32)  # [C_in, C_out]
    common = dict(
        xT=xTb,
        x_full=xb,
        WqT=np.ascontiguousarray(np.asarray(in_proj_w)[0:C].T.astype(BF)),
        WkT=np.ascontiguousarray(np.asarray(in_proj_w)[C:2 * C].T.astype(BF)),
        WvT=np.ascontiguousarray(np.asarray(in_proj_w)[2 * C:3 * C].T.astype(BF)),
        Wop4=np.ascontiguousarray(
            WopT.reshape(H, DH, C).transpose(1, 0, 2).reshape(DH, H * C).astype(BF)),
        Wl=np.ascontiguousarray(np.asarray(weight_local).astype(BF)),
        fcT=np.ascontiguousarray(np.asarray(fc_w).T.astype(BF)),
        bq_pack=np.ascontiguousarray(np.asarray(in_proj_b)[0:C].astype(np.float32).reshape(2, P).T),
        bk_pack=np.ascontiguousarray(np.asarray(in_proj_b)[C:2 * C].astype(np.float32).reshape(2, P).T),
        bv_rep=np.tile(np.asarray(in_proj_b)[2 * C:3 * C].astype(np.float32), (P, 1)),
        opb_rep=np.tile(np.asarray(out_proj_b, dtype=np.float32), (P, 1)),
        g_rep=np.tile(np.asarray(ln_g, dtype=np.float32), (P, 1)),
        b_rep=np.tile(np.asarray(ln_b, dtype=np.float32), (P, 1)),
        fcb_rep=np.tile(np.asarray(fc_b, dtype=np.float32), (P, 1)),
        alpha11=np.asarray(alpha, dtype=np.float32).reshape(1, 1),
        identb_in=np.eye(P, dtype=np.float32).astype(BF),
        ones_col_in=np.ones((P, 1), np.float32).astype(BF),
        ones_row_in=np.ones((1, P), dtype=np.float32),
    )
    in_maps = []
    for k in range(NCORES):
        ohg, ri = per_core_edges[k]
        m = dict(common)
        m['xT_own'] = np.ascontiguousarray(xTb[:, k * NPC:(k + 1) * NPC])
        m['x_own'] = np.ascontiguousarray(x[k * NPC:(k + 1) * NPC, :])
        m['oh_in'] = ohg
        m['row_idx'] = ri
        in_maps.append(m)

    nc = _build(TPT)
    res = run_bass_kernel_spmd(nc, in_maps, core_ids=list(range(NCORES)))
    LAST_RESULTS = res
    return np.concatenate([res.results[k]['out'] for k in range(NCORES)], axis=0)


# revision 24
# speedup vs baseline: 1.3829x; 1.1492x over previous
"""Trainium2 Bass kernel for LocalGlobalEnvEncoder (GCN + MHA fusion).

Sharding: nodes are split across the 8 cores (1024 dest nodes / queries each).
 - GCN: edges bucketed by destination node-tile on host (layout only); degrees
   computed on-device via one-hot matmuls, exchanged with an AllGather; messages
   gathered from a device-materialized y = x * rsqrt(d) table via batched
   indirect DMA and scatter-added with one-hot matmuls on the PE.
 - MHA: query-sharded attention, K/V computed redundantly per core, scores kept
   transposed ([key, query]); attn@V accumulates O^T per head with an appended
   ones-column in V providing softmax denominators.
Region A: degrees + QKV projections (PE busy while indices load).
Region B: AllGather + y table + attention + interleaved GCN scatter.
Region C: out_proj + LN + combine + fc.
All heavy matmuls run in bf16 (fp32 PSUM accumulation); normalization fp32.
"""
import sys
sys.path.insert(0, '/opt/trn_rl_repo')
import numpy as np
import ml_dtypes
import concourse.bass as bass
import concourse.tile as tile
from concourse import bacc, mybir, library_config
from concourse.bass_utils import run_bass_kernel_spmd

F32 = mybir.dt.float32
BF16 = mybir.dt.bfloat16
I32 = mybir.dt.int32
AF = mybir.ActivationFunctionType
OP = mybir.AluOpType
AX = mybir.AxisListType

N, E, C, OUTC, H, DH = 8192, 262144, 256, 256, 4, 64
NCORES = 8
NPC = N // NCORES          # nodes per core = 1024
P = 128
NT_LOC = NPC // P          # node tiles per core = 8
NT_GLOB = N // P           # global node tiles = 64
EXP_BIAS = -12.0           # uniform shift inside softmax exp; cancels in the ratio
GATHER_GRP = 8             # edge tiles per indirect-DMA gather (1024 descriptors)

LAST_RESULTS = None        # stashed BassKernelResults for test harness introspection


def _build(TPT):
    """Build the single SPMD Bass program. TPT = (deduped) edge tiles per
    node-tile segment. The one-hot array is padded to TPT_P (multiple of 8)
    so the wide degree matmuls stay aligned to dest-tile boundaries."""
    nc = bacc.Bacc('TRN2', target_bir_lowering=False, debug=False, num_devices=NCORES)
    TE = NT_LOC * TPT          # real edge tiles per core
    TPT_P = ((TPT + 7) // 8) * 8
    TE_P = NT_LOC * TPT_P      # padded (for the one-hot array only)

    # ---- I/O ----
    xT = nc.dram_tensor("xT", [C, N], BF16, kind="ExternalInput")
    xT_own = nc.dram_tensor("xT_own", [C, NPC], BF16, kind="ExternalInput")
    x_full = nc.dram_tensor("x_full", [N, C], BF16, kind="ExternalInput")
    x_own = nc.dram_tensor("x_own", [NPC, C], F32, kind="ExternalInput")
    WqT = nc.dram_tensor("WqT", [C, C], BF16, kind="ExternalInput")
    WkT = nc.dram_tensor("WkT", [C, C], BF16, kind="ExternalInput")
    WvT = nc.dram_tensor("WvT", [C, C], BF16, kind="ExternalInput")
    Wop4 = nc.dram_tensor("Wop4", [DH, H * C], BF16, kind="ExternalInput")
    Wl = nc.dram_tensor("Wl", [C, C], BF16, kind="ExternalInput")
    fcT = nc.dram_tensor("fcT", [C, OUTC], BF16, kind="ExternalInput")
    bq_pack = nc.dram_tensor("bq_pack", [P, 2], F32, kind="ExternalInput")
    bk_pack = nc.dram_tensor("bk_pack", [P, 2], F32, kind="ExternalInput")
    bv_rep = nc.dram_tensor("bv_rep", [P, C], F32, kind="ExternalInput")
    opb_rep = nc.dram_tensor("opb_rep", [P, C], F32, kind="ExternalInput")
    g_rep = nc.dram_tensor("g_rep", [P, C], F32, kind="ExternalInput")
    b_rep = nc.dram_tensor("b_rep", [P, C], F32, kind="ExternalInput")
    fcb_rep = nc.dram_tensor("fcb_rep", [P, OUTC], F32, kind="ExternalInput")
    alpha11 = nc.dram_tensor("alpha11", [1, 1], F32, kind="ExternalInput")
    identb_in = nc.dram_tensor("identb_in", [P, P], BF16, kind="ExternalInput")
    ones_col_in = nc.dram_tensor("ones_col_in", [P, 1], BF16, kind="ExternalInput")
    ones_row_in = nc.dram_tensor("ones_row_in", [1, P], F32, kind="ExternalInput")
    GOH = 8                    # one-hot tiles packed per DMA
    G_CNT = TE_P // GOH
    oh_in = nc.dram_tensor("oh_in", [G_CNT, P, GOH * P], BF16, kind="ExternalInput")
    row_idx = nc.dram_tensor("row_idx", [P, TE], I32, kind="ExternalInput")

    out = nc.dram_tensor("out", [NPC, OUTC], F32, kind="ExternalOutput")

    with tile.TileContext(nc) as tc:
        with tc.tile_pool(name="const", bufs=1) as const, \
             tc.tile_pool(name="big", bufs=1) as big, \
             tc.tile_pool(name="dram", bufs=1, space="DRAM") as dram:

            # ---- persistent constants ----
            identb_t = const.tile([P, P], BF16)
            nc.sync.dma_start(out=identb_t[:], in_=identb_in[:])
            ones_col_t = const.tile([P, 1], BF16)
            nc.sync.dma_start(out=ones_col_t[:], in_=ones_col_in[:])
            ones_row_t = const.tile([1, P], F32)
            nc.sync.dma_start(out=ones_row_t[:], in_=ones_row_in[:])
            row_t = const.tile([P, TE], I32)
            nc.sync.dma_start(out=row_t[:], in_=row_idx[:])
            expb_col = const.tile([P, 1], F32)
            nc.vector.memset(expb_col[:], EXP_BIAS)
            eps_col = const.tile([P, 1], F32)
            nc.vector.memset(eps_col[:], 1e-5)

            d_loc = const.tile([P, NT_LOC], F32)
            s_own = const.tile([P, NT_LOC], F32)
            s_all = const.tile([P, NT_GLOB], F32)
            w_col = const.tile([P, 1], F32)
            dg_in = dram.tile([NT_LOC, P], F32)
            dg_out = dram.tile([NT_GLOB, P], F32)
            y_d = dram.tile([N, C], BF16)
            den_dr = dram.tile([1, NPC], F32)

            # persistent SBUF state across regions
            KTp = [big.tile([P, N], BF16, name=f"KT{p}") for p in range(2)]
            QTp = [big.tile([P, NPC], BF16, name=f"QT{p}") for p in range(2)]
            Vt = big.tile([P, NT_GLOB * H * (DH + 1)], BF16, name="Vt")
            V4 = Vt[:].rearrange("p (k h d) -> p k h d", h=H, d=DH + 1)
            OTh = [big.tile([DH, NPC], BF16, name=f"OT{h}") for h in range(H)]
            hi_sb = [big.tile([P, C], BF16, name=f"hi{i}") for i in range(NT_LOC)]

            nc.vector.memset(V4[:, :, :, DH:DH + 1], 1.0)  # ones col -> denominators

            # ============ region A: degrees + sigmoid(alpha) + QKV ============
            with tc.tile_pool(name="phA", bufs=1) as phA, \
                 tc.tile_pool(name="psA", bufs=1, space="PSUM") as psA:
                # sigmoid(alpha) replicated to a [128,1] column
                al_t = phA.tile([1, 1], F32)
                nc.sync.dma_start(out=al_t[:], in_=alpha11[:])
                wsig = phA.tile([1, 1], F32)
                nc.scalar.activation(out=wsig[:], in_=al_t[:], func=AF.Sigmoid)
                wrep_ps = psA.tile([P, 1], F32, tag="wrep")
                nc.tensor.matmul(out=wrep_ps[:], lhsT=ones_row_t[:], rhs=wsig[:],
                                 start=True, stop=True)
                nc.vector.tensor_copy(out=w_col[:], in_=wrep_ps[:])

                # degrees of own nodes: colsums of the multiplicity one-hots
                # via wide-moving matmuls (ones column stationary), then a
                # tree fold of the 8 blocks and a transpose to column layout.
                NG_T = TPT_P // GOH        # one-hot groups per dest tile
                for t in range(NT_LOC):
                    dps = psA.tile([1, GOH * P], F32, tag="dps", bufs=1)
                    for gi in range(NG_T):
                        ohg = phA.tile([P, GOH * P], BF16, tag="oh", bufs=6,
                                       name=f"ohga{t}_{gi}")
                        nc.sync.dma_start(out=ohg[:], in_=oh_in[t * NG_T + gi, :, :])
                        for hf in range(2):
                            nc.tensor.matmul(
                                out=dps[0:1, hf * 512:(hf + 1) * 512],
                                lhsT=ones_col_t[:],
                                rhs=ohg[:, hf * 512:(hf + 1) * 512],
                                start=(gi == 0), stop=(gi == NG_T - 1))
                    dsr = phA.tile([1, GOH * P], BF16, tag="dsr", bufs=2)
                    nc.vector.tensor_copy(out=dsr[:], in_=dps[:])
                    w2 = GOH * P // 2
                    while w2 >= P:
                        nc.vector.tensor_tensor(out=dsr[0:1, 0:w2], in0=dsr[0:1, 0:w2],
                                                in1=dsr[0:1, w2:2 * w2], op=OP.add)
                        w2 //= 2
                    dtp = psA.tile([P, 1], BF16, tag="dtp", bufs=1)
                    nc.tensor.transpose(out=dtp[:], in_=dsr[0:1, 0:P],
                                        identity=identb_t[0:1, 0:1])
                    nc.vector.tensor_copy(out=d_loc[:, t:t + 1], in_=dtp[:])
                    nc.sync.dma_start(out=dg_in[t, :, None], in_=d_loc[:, t:t + 1])

                # launch the degree exchange early; it overlaps the QKV work
                nc.gpsimd.collective_compute(
                    "AllGather", OP.bypass,
                    replica_groups=[list(range(NCORES))],
                    ins=[dg_in[:].opt()], outs=[dg_out[:].opt()])

                # rsqrt for own nodes (s_own); s_all comes after the AllGather
                m_t = phA.tile([P, NT_LOC], F32, tag="mo")
                nc.vector.tensor_scalar(out=m_t[:], in0=d_loc[:], scalar1=1.0,
                                        scalar2=None, op0=OP.min)
                t1 = phA.tile([P, NT_LOC], F32, tag="t1o")
                nc.vector.tensor_scalar(out=t1[:], in0=d_loc[:], scalar1=1.0,
                                        scalar2=None, op0=OP.add)
                nc.vector.tensor_tensor(out=t1[:], in0=t1[:], in1=m_t[:], op=OP.subtract)
                nc.scalar.activation(out=t1[:], in_=t1[:], func=AF.Sqrt)
                nc.vector.reciprocal(out=t1[:], in_=t1[:])
                nc.vector.tensor_tensor(out=s_own[:], in0=t1[:], in1=m_t[:], op=OP.mult)

                def emit_rsqrt_all_and_y(pool):
                    d_all = pool.tile([P, NT_GLOB], F32, name="d_all")
                    nc.sync.dma_start(out=d_all[:], in_=dg_out[:].rearrange("g p -> p g"))
                    m_a = pool.tile([P, NT_GLOB], F32, name="m_a")
                    nc.vector.tensor_scalar(out=m_a[:], in0=d_all[:], scalar1=1.0,
                                            scalar2=None, op0=OP.min)
                    t1a = pool.tile([P, NT_GLOB], F32, name="t1a")
                    nc.vector.tensor_scalar(out=t1a[:], in0=d_all[:], scalar1=1.0,
                                            scalar2=None, op0=OP.add)
                    nc.vector.tensor_tensor(out=t1a[:], in0=t1a[:], in1=m_a[:],
                                            op=OP.subtract)
                    nc.scalar.activation(out=t1a[:], in_=t1a[:], func=AF.Sqrt)
                    nc.vector.reciprocal(out=t1a[:], in_=t1a[:])
                    nc.vector.tensor_tensor(out=s_all[:], in0=t1a[:], in1=m_a[:],
                                            op=OP.mult)
                    # y = x * rsqrt(d)[node] (node-major, written to DRAM),
                    # 4 node-tiles per DMA to keep queue occupancy low
                    GY = 4
                    for g4 in range(NT_GLOB // GY):
                        xt = pool.tile([P, GY, C], BF16, tag="xt", bufs=3)
                        nc.scalar.dma_start(
                            out=xt[:],
                            in_=x_full[g4 * GY * P:(g4 + 1) * GY * P, :]
                                .rearrange("(t p) c -> p t c", p=P))
                        yt = pool.tile([P, GY, C], BF16, tag="yt", bufs=3)
                        for u in range(GY):
                            nc.vector.tensor_scalar(
                                out=yt[:, u, :], in0=xt[:, u, :],
                                scalar1=s_all[:, g4 * GY + u:g4 * GY + u + 1],
                                scalar2=None, op0=OP.mult)
                        nc.sync.dma_start(
                            out=y_d[g4 * GY * P:(g4 + 1) * GY * P, :]
                                .rearrange("(t p) c -> p t c", p=P),
                            in_=yt[:])

                # ---- QKV projections (independent of degrees; keeps PE busy) ----
                Wq_t = phA.tile([P, 2 * C], BF16)
                nc.sync.dma_start(out=Wq_t[:].rearrange("p (c n) -> p c n", c=2), in_=WqT[:].rearrange("(c p) n -> p c n", p=P))
                Wk_t = phA.tile([P, 2 * C], BF16)
                nc.sync.dma_start(out=Wk_t[:].rearrange("p (c n) -> p c n", c=2), in_=WkT[:].rearrange("(c p) n -> p c n", p=P))
                Wv_t = phA.tile([P, 2 * C], BF16)
                nc.sync.dma_start(out=Wv_t[:].rearrange("p (c n) -> p c n", c=2), in_=WvT[:].rearrange("(c p) n -> p c n", p=P))
                bq_t = phA.tile([P, 2], F32)
                nc.sync.dma_start(out=bq_t[:], in_=bq_pack[:])
                bk_t = phA.tile([P, 2], F32)
                nc.sync.dma_start(out=bk_t[:], in_=bk_pack[:])
                bv_t = phA.tile([P, C], F32)
                nc.sync.dma_start(out=bv_t[:], in_=bv_rep[:])

                # Q from xT_own
                xo = [phA.tile([P, NPC], BF16, tag=f"xo{c}", name=f"xo{c}") for c in range(2)]
                for c in range(2):
                    nc.sync.dma_start(out=xo[c][:], in_=xT_own[c * P:(c + 1) * P, :])
                for p in range(2):
                    for nb in range(NPC // 512):
                        qps = psA.tile([P, 512], F32, tag="qkps", bufs=2)
                        for c in range(2):
                            nc.tensor.matmul(
                                out=qps[:],
                                lhsT=Wq_t[:, c * C + p * P: c * C + (p + 1) * P],
                                rhs=xo[c][:, nb * 512:(nb + 1) * 512],
                                start=(c == 0), stop=(c == 1))
                        nc.vector.tensor_scalar(
                            out=QTp[p][:, nb * 512:(nb + 1) * 512], in0=qps[:],
                            scalar1=bq_t[:, p:p + 1], scalar2=None, op0=OP.add)

                # K and V in slabs of 1024 nodes
                SLAB = 1024
                for s in range(N // SLAB):
                    xts = [phA.tile([P, SLAB], BF16, tag=f"xts{c}", bufs=2, name=f"xts{c}_{s}") for c in range(2)]
                    for c in range(2):
                        nc.sync.dma_start(out=xts[c][:],
                                          in_=xT[c * P:(c + 1) * P, s * SLAB:(s + 1) * SLAB])
                    for p in range(2):
                        for nb in range(SLAB // 512):
                            kps = psA.tile([P, 512], F32, tag="qkps", bufs=2)
                            for c in range(2):
                                nc.tensor.matmul(
                                    out=kps[:],
                                    lhsT=Wk_t[:, c * C + p * P: c * C + (p + 1) * P],
                                    rhs=xts[c][:, nb * 512:(nb + 1) * 512],
                                    start=(c == 0), stop=(c == 1))
                            nc.vector.tensor_scalar(
                                out=KTp[p][:, s * SLAB + nb * 512: s * SLAB + (nb + 1) * 512],
                                in0=kps[:], scalar1=bk_t[:, p:p + 1], scalar2=None,
                                op0=OP.add)
                    for ntl in range(SLAB // P):
                        g = s * (SLAB // P) + ntl
                        vps = psA.tile([P, C], F32, tag="vps", bufs=2)
                        for c in range(2):
                            nc.tensor.matmul(
                                out=vps[:],
                                lhsT=xts[c][:, ntl * P:(ntl + 1) * P],
                                rhs=Wv_t[:, c * C:(c + 1) * C],
                                start=(c == 0), stop=(c == 1))
                        nc.vector.tensor_tensor(
                            out=V4[:, g, :, 0:DH],
                            in0=vps[:].rearrange("p (h d) -> p h d", d=DH),
                            in1=bv_t[:].rearrange("p (h d) -> p h d", d=DH),
                            op=OP.add)

                # collective has been running since before QKV; finish the
                # degree math and materialize the y table.
                emit_rsqrt_all_and_y(phA)

            # == region B: attention + GCN scatter ==
            with tc.tile_pool(name="phB", bufs=1) as phB, \
                 tc.tile_pool(name="psB", bufs=1, space="PSUM") as psB:

                # GCN scatter: single-column indirect gathers (DGE prep is
                # ~1us each on the gpsimd queue -- the pacing constraint), with
                # one-hot matmuls trailing the gathers by a prefetch margin.
                n_gath = NT_LOC * TPT
                yg_tiles = {}
                g_emitted = 0
                mm_emitted = 0
                hips_cur = {}
                ohg_b = {}

                def emit_gathers(upto):
                    nonlocal g_emitted
                    while g_emitted < min(upto, n_gath):
                        j = g_emitted
                        yg = phB.tile([P, C], BF16, tag="yg", bufs=40,
                                      name=f"yg{j}")
                        nc.gpsimd.indirect_dma_start(
                            out=yg[:], out_offset=None, in_=y_d[:],
                            in_offset=bass.IndirectOffsetOnAxis(
                                ap=row_t[:, j:j + 1], axis=0))
                        yg_tiles[j] = yg
                        g_emitted += 1

                def emit_mms(upto):
                    nonlocal mm_emitted
                    while mm_emitted < min(upto, n_gath):
                        j = mm_emitted
                        t, i = j // TPT, j % TPT
                        yg = yg_tiles.pop(j)
                        if i == 0:
                            hips_cur[t] = psB.tile([P, C], F32, tag="hips", bufs=2,
                                                   name=f"hips{t}")
                        jp = t * TPT_P + i
                        if jp % GOH == 0:
                            ohg_b[jp // GOH] = phB.tile([P, GOH * P], BF16,
                                                        tag="oh2", bufs=8,
                                                        name=f"ohgb{jp}")
                            nc.sync.dma_start(out=ohg_b[jp // GOH][:],
                                              in_=oh_in[jp // GOH, :, :])
                        oh = ohg_b[jp // GOH][:, (jp % GOH) * P:(jp % GOH + 1) * P]
                        nc.tensor.matmul(out=hips_cur[t][:], lhsT=oh, rhs=yg[:],
                                         start=(i == 0), stop=(i == TPT - 1))
                        if i == TPT - 1:
                            nc.vector.tensor_scalar(out=hi_sb[t][:], in0=hips_cur[t][:],
                                                    scalar1=s_own[:, t:t + 1],
                                                    scalar2=None, op0=OP.mult)
                        mm_emitted += 1

                # attention: scores -> exp -> O^T accumulation (2 steps behind)
                G_T0, G_T1 = 0, 232      # gather emission window (steps)
                M_T0, M_T1 = 16, 248     # scatter-matmul emission window
                step = 0

                def maybe_scatter():
                    if step >= G_T0:
                        emit_gathers(1 + n_gath * (step - G_T0) // (G_T1 - G_T0))
                    if step >= M_T0:
                        emit_mms(1 + n_gath * (step - M_T0) // (M_T1 - M_T0))

                et_t = {}

                def emit_scores(h, kt):
                    p, hh = h // 2, h % 2
                    po = hh * DH
                    sps = psB.tile([P, NPC], F32, tag="sps", bufs=2)
                    for qh in range(2):
                        nc.tensor.matmul(
                            out=sps[:, qh * 512:(qh + 1) * 512],
                            lhsT=KTp[p][po:po + DH, kt * P:(kt + 1) * P],
                            rhs=QTp[p][po:po + DH, qh * 512:(qh + 1) * 512],
                            start=True, stop=True)
                    et = phB.tile([P, NPC], BF16, tag="expT", bufs=3)
                    nc.scalar.activation(out=et[:], in_=sps[:], func=AF.Exp,
                                         bias=expb_col[:, 0:1], scale=1.0 / np.sqrt(DH))
                    et_t[(h, kt)] = et

                def emit_attnv(h, kt, OpsT):
                    et = et_t.pop((h, kt))
                    for qh in range(2):
                        nc.tensor.matmul(
                            out=OpsT[:, qh * 512:(qh + 1) * 512],
                            lhsT=V4[:, kt, h, :],
                            rhs=et[:, qh * 512:(qh + 1) * 512],
                            start=(kt == 0), stop=(kt == NT_GLOB - 1))

                OpsT_h = {}
                pend = []            # (h, kt) with scores emitted, attnV pending
                for h in range(H):
                    OpsT_h[h] = psB.tile([DH + 1, NPC], F32, tag="OpsT", bufs=1,
                                         name=f"OpsT{h}")
                    for kt in range(NT_GLOB):
                        emit_scores(h, kt)
                        pend.append((h, kt))
                        if len(pend) > 2:
                            ph_, pk_ = pend.pop(0)
                            emit_attnv(ph_, pk_, OpsT_h[ph_])
                        step += 1
                        maybe_scatter()
                    # finish this head's attnV before normalization
                    while pend:
                        ph_, pk_ = pend.pop(0)
                        emit_attnv(ph_, pk_, OpsT_h[ph_])
                    # ---- normalize O^T rows by the denominator row ----
                    # (den broadcast across partitions goes through DRAM: SBUF
                    # sources cannot have a zero partition step, DRAM can.)
                    OpsT = OpsT_h.pop(h)
                    Osb = phB.tile([DH + 1, NPC], F32, tag="Osb", bufs=2)
                    nc.vector.tensor_copy(out=Osb[:], in_=OpsT[:])
                    nc.sync.dma_start(out=den_dr[:], in_=Osb[DH:DH + 1, :])
                    bcast = phB.tile([DH, NPC], F32, tag="bcast", bufs=2)
                    nc.sync.dma_start(out=bcast[:], in_=den_dr[:].to_broadcast((DH, NPC)))
                    nc.vector.reciprocal(out=bcast[:], in_=bcast[:])
                    nc.vector.tensor_tensor(out=OTh[h][:], in0=Osb[0:DH, :],
                                            in1=bcast[:], op=OP.mult)
                emit_gathers(n_gath)
                emit_mms(n_gath)

            # ========= region C: out_proj, LN, combine, fc =========
            with tc.tile_pool(name="phC", bufs=1) as phC, \
                 tc.tile_pool(name="psC", bufs=1, space="PSUM") as psC:
                Wop_t = phC.tile([DH, H * C], BF16)
                nc.sync.dma_start(out=Wop_t[:], in_=Wop4[:])
                Wl_t = phC.tile([P, 2 * C], BF16)
                nc.sync.dma_start(out=Wl_t[:].rearrange("p (c n) -> p c n", c=2), in_=Wl[:].rearrange("(c p) n -> p c n", p=P))
                fc_t = phC.tile([P, 2 * OUTC], BF16)
                nc.sync.dma_start(out=fc_t[:].rearrange("p (c n) -> p c n", c=2), in_=fcT[:].rearrange("(c p) n -> p c n", p=P))
                opb_t = phC.tile([P, C], F32)
                nc.sync.dma_start(out=opb_t[:], in_=opb_rep[:])
                g_t = phC.tile([P, C], F32)
                nc.sync.dma_start(out=g_t[:], in_=g_rep[:])
                b_t = phC.tile([P, C], F32)
                nc.sync.dma_start(out=b_t[:], in_=b_rep[:])
                fcb_t = phC.tile([P, OUTC], F32)
                nc.sync.dma_start(out=fcb_t[:], in_=fcb_rep[:])

                def transpose_2chunks(src_ap, tag):
                    dst = phC.tile([P, C], BF16, tag=tag, bufs=2)
                    for c in range(2):
                        tp = psC.tile([P, P], BF16, tag="tp", bufs=2)
                        nc.tensor.transpose(out=tp[:], in_=src_ap[:, c * P:(c + 1) * P],
                                            identity=identb_t[:])
                        nc.vector.tensor_copy(out=dst[:, c * P:(c + 1) * P], in_=tp[:])
                    return dst

                for qt in range(NT_LOC):
                    # ---- global path: out_proj (per-head, 64-contract) + LN ----
                    aps = psC.tile([P, C], F32, tag="aps", bufs=2)
                    for h in range(H):
                        nc.tensor.matmul(out=aps[:],
                                         lhsT=OTh[h][:, qt * P:(qt + 1) * P],
                                         rhs=Wop_t[:, h * C:(h + 1) * C],
                                         start=(h == 0), stop=(h == H - 1))
                    v_t = phC.tile([P, C], F32, tag="vt", bufs=2)
                    nc.vector.tensor_tensor(out=v_t[:], in0=aps[:], in1=opb_t[:], op=OP.add)
                    xo_t = phC.tile([P, C], F32, tag="xot", bufs=2)
                    nc.sync.dma_start(out=xo_t[:], in_=x_own[qt * P:(qt + 1) * P, :])
                    nc.vector.tensor_tensor(out=v_t[:], in0=v_t[:], in1=xo_t[:], op=OP.add)
                    # LN stats on ACT via accum_out; elementwise tail on DVE
                    scr = phC.tile([P, C], F32, tag="scr", bufs=2)
                    msum = phC.tile([P, 1], F32, tag="msum", bufs=2)
                    nc.scalar.activation(out=scr[:], in_=v_t[:], func=AF.Identity,
                                         accum_out=msum[:])
                    nmean = phC.tile([P, 1], F32, tag="nmean", bufs=2)
                    nc.scalar.activation(out=nmean[:], in_=msum[:], func=AF.Copy,
                                         scale=-1.0 / C)
                    ssum = phC.tile([P, 1], F32, tag="ssum", bufs=2)
                    nc.scalar.activation(out=v_t[:], in_=v_t[:], func=AF.Identity,
                                         bias=nmean[:, 0:1])
                    nc.scalar.activation(out=scr[:], in_=v_t[:], func=AF.Square,
                                         accum_out=ssum[:])
                    sstd = phC.tile([P, 1], F32, tag="sstd", bufs=2)
                    nc.scalar.activation(out=sstd[:], in_=ssum[:], func=AF.Sqrt,
                                         bias=eps_col[:, 0:1], scale=1.0 / C)
                    rstd = phC.tile([P, 1], F32, tag="rstd", bufs=2)
                    nc.vector.reciprocal(out=rstd[:], in_=sstd[:])
                    nc.scalar.activation(out=v_t[:], in_=v_t[:], func=AF.Identity,
                                         scale=rstd[:, 0:1])
                    nc.vector.tensor_tensor(out=v_t[:], in0=v_t[:], in1=g_t[:], op=OP.mult)
                    nc.vector.tensor_tensor(out=v_t[:], in0=v_t[:], in1=b_t[:], op=OP.add)

                    # ---- local path: local_embed = hi @ W_local ----
                    hiT = transpose_2chunks(hi_sb[qt][:], "hiT")
                    lps = psC.tile([P, C], F32, tag="lps", bufs=2)
                    for c in range(2):
                        nc.tensor.matmul(out=lps[:], lhsT=hiT[:, c * P:(c + 1) * P],
                                         rhs=Wl_t[:, c * C:(c + 1) * C],
                                         start=(c == 0), stop=(c == 1))
                    # combined = global + w * (local - global)
                    comb = phC.tile([P, C], F32, tag="comb", bufs=2)
                    nc.vector.tensor_tensor(out=comb[:], in0=lps[:], in1=v_t[:],
                                            op=OP.subtract)
                    nc.vector.tensor_scalar(out=comb[:], in0=comb[:], scalar1=w_col[:, 0:1],
                                            scalar2=None, op0=OP.mult)
                    nc.vector.tensor_tensor(out=comb[:], in0=comb[:], in1=v_t[:], op=OP.add)

                    # ---- fc ----
                    comb_b = phC.tile([P, C], BF16, tag="combb", bufs=2)
                    nc.vector.tensor_copy(out=comb_b[:], in_=comb[:])
                    cT = transpose_2chunks(comb_b[:], "cT")
                    fps = psC.tile([P, OUTC], F32, tag="fps", bufs=2)
                    for c in range(2):
                        nc.tensor.matmul(out=fps[:], lhsT=cT[:, c * P:(c + 1) * P],
                                         rhs=fc_t[:, c * OUTC:(c + 1) * OUTC],
                                         start=(c == 0), stop=(c == 1))
                    o_t = phC.tile([P, OUTC], F32, tag="ot", bufs=2)
                    nc.vector.tensor_tensor(out=o_t[:], in0=fps[:], in1=fcb_t[:], op=OP.add)
                    nc.sync.dma_start(out=out[qt * P:(qt + 1) * P, :], in_=o_t[:])
    nc.finalize()
    return nc


def _prep_edges(adj):
    """Bucket edges by destination node-tile; dedupe source rows per segment.

    For each dest tile the distinct source rows are gathered once and the
    one-hot matrix carries edge multiplicities (a re-encoding of the
    adjacency list -- layout only). Returns per-core
    (ohg [G_CNT, P, GOH*P] bf16, row_idx [P, TE] int32) and TPT.
    """
    BF = ml_dtypes.bfloat16
    GOH = 8
    row = np.asarray(adj[0], dtype=np.int64)
    col = np.asarray(adj[1], dtype=np.int64)
    tid = col // P
    order = np.argsort(tid, kind='stable')
    row_s, col_s = row[order], col[order]
    counts = np.bincount(tid, minlength=NT_GLOB)
    bounds = np.concatenate([[0], np.cumsum(counts)])
    seg = []
    max_distinct = 1
    for g in range(NT_GLOB):
        r = row_s[bounds[g]:bounds[g + 1]]
        c = (col_s[bounds[g]:bounds[g + 1]] - g * P).astype(np.int64)
        uniq, inv = np.unique(r, return_inverse=True)
        seg.append((uniq, inv, c))
        max_distinct = max(max_distinct, len(uniq))
    S = int(np.ceil(max_distinct / P) * P)
    TPT = S // P
    TPT_P = ((TPT + 7) // 8) * 8
    per_core = []
    for k in range(NCORES):
        oh = np.zeros((NT_LOC, TPT_P * P, P), np.float32)
        ri = np.zeros((NT_LOC, S), np.int32)
        for t in range(NT_LOC):
            uniq, inv, c = seg[NT_LOC * k + t]
            np.add.at(oh[t], (inv, c), 1.0)
            ri[t, :len(uniq)] = uniq.astype(np.int32)
        # pack GOH one-hot tiles side by side in the free dim
        ohg = oh.reshape(NT_LOC * TPT_P // GOH, GOH, P, P).transpose(0, 2, 1, 3)
        ohg = np.ascontiguousarray(
            ohg.reshape(NT_LOC * TPT_P // GOH, P, GOH * P).astype(BF))
        rit = np.ascontiguousarray(ri.reshape(NT_LOC * TPT, P).T.astype(np.int32))
        per_core.append((ohg, rit))
    return per_core, TPT


def kernel(x, adj, weight_local, in_proj_w, in_proj_b, out_proj_w, out_proj_b,
           ln_g, ln_b, alpha, fc_w, fc_b):
    global LAST_RESULTS
    BF = ml_dtypes.bfloat16
    x = np.ascontiguousarray(np.asarray(x, dtype=np.float32))
    per_core_edges, TPT = _prep_edges(np.asarray(adj))

    xb = x.astype(BF)
    xTb = np.ascontiguousarray(xb.T)
    WopT = np.asarray(out_proj_w).T.astype(np.float32)  # [C_in, C_out]
    common = dict(
        xT=xTb,
        x_full=xb,
        WqT=np.ascontiguousarray(np.asarray(in_proj_w)[0:C].T.astype(BF)),
        WkT=np.ascontiguousarray(np.asarray(in_proj_w)[C:2 * C].T.astype(BF)),
        WvT=np.ascontiguousarray(np.asarray(in_proj_w)[2 * C:3 * C].T.astype(BF)),
        Wop4=np.ascontiguousarray(
            WopT.reshape(H, DH, C).transpose(1, 0, 2).reshape(DH, H * C).astype(BF)),
        Wl=np.ascontiguousarray(np.asarray(weight_local).astype(BF)),
        fcT=np.ascontiguousarray(np.asarray(fc_w).T.astype(BF)),
        bq_pack=np.ascontiguousarray(np.asarray(in_proj_b)[0:C].astype(np.float32).reshape(2, P).T),
        bk_pack=np.ascontiguousarray(np.asarray(in_proj_b)[C:2 * C].astype(np.float32).reshape(2, P).T),
        bv_rep=np.tile(np.asarray(in_proj_b)[2 * C:3 * C].astype(np.float32), (P, 1)),
        opb_rep=np.tile(np.asarray(out_proj_b, dtype=np.float32), (P, 1)),
        g_rep=np.tile(np.asarray(ln_g, dtype=np.float32), (P, 1)),
        b_rep=np.tile(np.asarray(ln_b, dtype=np.float32), (P, 1)),
        fcb_rep=np.tile(np.asarray(fc_b, dtype=np.float32), (P, 1)),
        alpha11=np.asarray(alpha, dtype=np.float32).reshape(1, 1),
        identb_in=np.eye(P, dtype=np.float32).astype(BF),
        ones_col_in=np.ones((P, 1), np.float32).astype(BF),
        ones_row_in=np.ones((1, P), dtype=np.float32),
    )
    in_maps = []
    for k in range(NCORES):
        ohg, ri = per_core_edges[k]
        m = dict(common)
        m['xT_own'] = np.ascontiguousarray(xTb[:, k * NPC:(k + 1) * NPC])
        m['x_own'] = np.ascontiguousarray(x[k * NPC:(k + 1) * NPC, :])
        m['oh_in'] = ohg
        m['row_idx'] = ri
        in_maps.append(m)

    nc = _build(TPT)
    res = run_bass_kernel_spmd(nc, in_maps, core_ids=list(range(NCORES)))
    LAST_RESULTS = res
    return np.concatenate([res.results[k]['out'] for k in range(NCORES)], axis=0)
